# revision 1
# baseline (speedup 1.0000x reference)
"""Differential attention (nn_DifferentialAttention_84679575208071) on 8 TRN2
NeuronCores via Bass/Tile.

Sharding: hybrid data-parallel x tensor-parallel. Core c handles batch c//4 and
heads 4*(c%4) .. 4*(c%4)+4 (B=2, H=16 -> 2 batch groups x 4-way head split).
Each core computes its 4 heads' attention + group-norm + partial o_proj; the
host sums the 4 partial (S, D) outputs per batch (tensor-parallel o_proj
reduction) and stacks the 2 batches.

Per-core algorithm (all matmuls f32r = full-rate PE):
  xT = x^T (PE transposes)                        [D, S]
  qT/kT = (Wq/Wk slice)^T x^T with RoPE applied   [512, S] (2 branches x 4 heads x 64)
  v    = x Wv slice (row layout)                  [S, 256], augmented with a
         ones column per head -> PV matmul also yields softmax denominators
  attention, chunk-major: per (sq-chunk, head): both branches' score tiles go
         into one [128,1024] PSUM -> one wide exp (no max subtraction; scores
         are O(+-6) so fp32 exp is exact-safe) -> two PV matmuls [V|1]^T E^T
  diff = O1 - (lam*r1/r2) * O2  (group-norm is scale-invariant up to eps, so
         normalizing by r1 is folded in; eps is scaled by r1^2 to compensate)
  group-norm over each head's 64 channels, * gn_weight*(1-lam_init) + bias,
         then o_proj — both streamed per chunk so they overlap the next
         chunk's attention
"""
import json
import time

import numpy as np

import bass_rust
import concourse.bass as bass
import concourse.mybir as mybir
import concourse.tile as tile

F32 = mybir.dt.float32
F32R = mybir.dt.float32r
BF16 = mybir.dt.bfloat16
MULT = mybir.AluOpType.mult
ADD = mybir.AluOpType.add
SUB = mybir.AluOpType.subtract
AX = mybir.AxisListType.X
AF = mybir.ActivationFunctionType

B, S, D = 2, 2048, 1024
H, Hd = 16, 64
HPC = 4               # heads per core
THDC = HPC * Hd       # 256 channels per core
NT = S // 128         # 16 s-tiles
NCH = S // 512        # 4 chunks
DK = D // 128         # 8 contraction tiles
LAMBDA_INIT = 0.8
GN_EPS = 1e-5
ROPE_BASE = 10000.0
QK_SCALE = float(Hd) ** -0.5  # 0.125


def _split_multi_waits(nc):
    """This container's walrus rejects >1 sync wait per instruction. Hoist
    extra waits onto same-engine NoOps inserted right before the instruction
    (engine queues are in-order, so semantics are unchanged)."""
    d = json.loads(bass_rust.module_to_json_string(nc.m))
    ctr = 0
    for f in d["functions"]:
        for bb in f["blocks"]:
            out = []
            for inst in bb.get("instructions", []):
                si = inst.get("sync_info")
                waits = si.get("on_wait", []) if si else []
                if len(waits) > 1:
                    for w in waits[:-1]:
                        ctr += 1
                        out.append({
                            "debug": inst.get("debug", 0),
                            "engine": inst["engine"],
                            "ins": [],
                            "name": f"WSPLIT-{ctr}",
                            "opcode": "NoOp",
                            "outs": [],
                            "sync_info": {"on_update": [], "on_wait": [w]},
                        })
                    si["on_wait"] = [waits[-1]]
                out.append(inst)
            bb["instructions"] = out
    nc.m = bass_rust.module_from_json_string(json.dumps(d))


def _rope_tables():
    inv_freq = 1.0 / (ROPE_BASE ** (np.arange(0, Hd, 2, dtype=np.float32) / Hd))
    t = np.arange(S, dtype=np.float32)
    freqs = np.outer(t, inv_freq).astype(np.float32)
    emb = np.concatenate([freqs, freqs], axis=-1)       # (S, Hd)
    cos = np.cos(emb).astype(np.float32)
    sin = np.sin(emb).astype(np.float32)
    return cos.T.copy(), sin.T.copy()                   # (Hd, S) each


def _hview(t):
    """[128, 260] tile -> [128, 4, 64] per-head data columns."""
    return t[:, :].rearrange("p (h c) -> p h c", h=HPC)[:, :, 0:Hd]


def _rview(t):
    """[128, 260] tile -> [128, 4] per-head r (denominator) columns."""
    v = t[:, :].rearrange("p (h c) -> p h c", h=HPC)[:, :, Hd:Hd + 1]
    return v.rearrange("p h c -> p (h c)")


def _b64(ap):
    """[128, 4] -> [128, 4, 64] broadcast along a new inner dim."""
    return ap.unsqueeze(2).broadcast_to((128, HPC, Hd))


def build_module():
    nc = bass.Bass(trn_type="TRN2")

    x_d = nc.dram_tensor("x", [S, D], F32, kind="ExternalInput")
    wq_d = nc.dram_tensor("wq", [D, 512], F32, kind="ExternalInput")
    wk_d = nc.dram_tensor("wk", [D, 512], F32, kind="ExternalInput")
    wv_d = nc.dram_tensor("wv", [D, THDC], F32, kind="ExternalInput")
    wo_d = nc.dram_tensor("wo", [THDC, D], F32, kind="ExternalInput")
    lam_d = nc.dram_tensor("lam", [5, HPC], F32, kind="ExternalInput")
    gnw_d = nc.dram_tensor("gnw", [THDC], F32, kind="ExternalInput")
    gnb_d = nc.dram_tensor("gnb", [THDC], F32, kind="ExternalInput")
    cos_d = nc.dram_tensor("cosT", [Hd, S], F32, kind="ExternalInput")
    sin_d = nc.dram_tensor("ssinT", [Hd, S], F32, kind="ExternalInput")
    y_d = nc.dram_tensor("y", [S, D], F32, kind="ExternalOutput")
    ident_d = nc.inline_tensor(np.eye(128, dtype=np.float32), name="identity")
    # rotate_half as a 128x128 +-1 permutation (block-diag per 64-row head):
    # perm @ q gives rh(q); transposed for the lhsT slot
    _m64 = np.zeros((64, 64), dtype=np.float32)
    for _d in range(32):
        _m64[_d, _d + 32] = -1.0
        _m64[_d + 32, _d] = 1.0
    _m128 = np.zeros((128, 128), dtype=np.float32)
    _m128[:64, :64] = _m64
    _m128[64:, 64:] = _m64
    rotm_d = nc.inline_tensor(np.ascontiguousarray(_m128.T), name="rotm")

    dma = nc.sync.dma_start  # HWDGE: no Pool-engine descriptor-prep cost

    with tile.TileContext(nc) as tc:
        with tc.tile_pool(name="persist", bufs=1) as pA, \
             tc.tile_pool(name="dscr", bufs=1, space="DRAM") as pD:
            qt = [pA.tile([128, S], BF16, tag=f"qt{i}", name=f"qt{i}") for i in range(4)]
            kt = [pA.tile([128, S], BF16, tag=f"kt{i}", name=f"kt{i}") for i in range(4)]
            vaug = [pA.tile([128, HPC * (Hd + 1)], F32R, tag=f"va{i}", name=f"va{i}")
                    for i in range(NT)]
            ident = pA.tile([128, 128], F32, tag="ident", name="ident")
            dma(out=ident, in_=ident_d.ap())
            rotm_raw = pA.tile([128, 128], F32, tag="rotm_raw", name="rotm_raw")
            dma(out=rotm_raw, in_=rotm_d.ap())
            rotm = pA.tile([128, 128], F32R, tag="rotm", name="rotm")
            nc.vector.tensor_copy(out=rotm, in_=rotm_raw)

            ones1 = pA.tile([128, 1], F32, tag="ones1", name="ones1")
            nc.vector.memset(ones1, 1.0)

            # ---- phases 1-2: x^T (full S), V/K/Q projections, rope ----
            with tc.tile_pool(name="ph12", bufs=1) as pB, \
                 tc.tile_pool(name="xload", bufs=4) as pX, \
                 tc.tile_pool(name="wraw", bufs=4) as pWr, \
                 tc.tile_pool(name="wtile", bufs=34) as pW, \
                 tc.tile_pool(name="wvrow", bufs=8) as pWv, \
                 tc.tile_pool(name="qsh", bufs=2) as pQs, \
                 tc.tile_pool(name="trot", bufs=2) as pT, \
                 tc.tile_pool(name="ps_tr", bufs=4, space="PSUM") as pPtr, \
                 tc.tile_pool(name="ps_proj", bufs=3, space="PSUM") as pPproj:
                cos2h = pB.tile([128, S], F32, tag="cos2h", name="cos2h")
                sin2h = pB.tile([128, S], F32, tag="sin2h", name="sin2h")
                xT = [pB.tile([128, S], F32R, tag=f"xT{i}", name=f"xT{i}")
                      for i in range(DK)]

                def rope(dst, tt_i, c, psq):
                    # One ACT copy stages psq -> SBUF; the rotate_half row
                    # permutation runs on the PE as a +-1 matmul; DVE then does
                    # cos-mul (2x from SBUF), sin-mul (1x from PSUM), and add.
                    cs = slice(c * 512, (c + 1) * 512)
                    qstage = pQs.tile([128, 512], F32R, tag="qstage", name=f"qst{tt_i}_{c}")
                    nc.scalar.copy(out=qstage, in_=psq)
                    perm = pPtr.tile([128, 512], F32, tag="ptr", name=f"perm{tt_i}_{c}")
                    nc.tensor.matmul(perm, lhsT=rotm, rhs=qstage, start=True, stop=True)
                    nc.vector.tensor_tensor(out=dst[tt_i][:, cs], in0=qstage,
                                            in1=cos2h[:, cs], op=MULT)
                    trot = pT.tile([128, 512], F32, tag="trot", name=f"trot{tt_i}_{c}")
                    nc.vector.tensor_tensor(out=trot, in0=perm, in1=sin2h[:, cs], op=MULT)
                    nc.vector.tensor_tensor(out=dst[tt_i][:, cs], in0=dst[tt_i][:, cs],
                                            in1=trot, op=ADD)

                # x^T: per s-tile, two half loads + 8 transposes
                for st in range(NT):
                    halves = []
                    for hf in range(2):
                        xtile = pX.tile([128, 512], F32, tag="xtile", name=f"xt{st}_{hf}")
                        dma(out=xtile, in_=x_d[st * 128:(st + 1) * 128,
                                              hf * 512:(hf + 1) * 512])
                        halves.append(xtile)
                    if st == 1:
                        # rope tables: emitted after the first x loads so they
                        # don't delay the critical first transposes
                        dma(out=cos2h[0:64, :], in_=cos_d[:, :])
                        dma(out=cos2h[64:128, :], in_=cos_d[:, :])
                        dma(out=sin2h[0:64, :], in_=sin_d[:, :])
                        dma(out=sin2h[64:128, :], in_=sin_d[:, :])
                    for dk in range(DK):
                        ptr = pPtr.tile([128, 128], F32, tag="ptr", name=f"ptx{st}_{dk}")
                        nc.tensor.transpose(ptr, halves[dk // 4][:, (dk % 4) * 128:
                                                                  (dk % 4 + 1) * 128], ident)
                        nc.scalar.copy(out=xT[dk][:, st * 128:(st + 1) * 128], in_=ptr)

                # V projection first (attention is gated on it)
                wvrow = []
                for dk in range(DK):
                    raw = pWr.tile([128, THDC], F32, tag="wvraw", name=f"wvr{dk}")
                    dma(out=raw, in_=wv_d[dk * 128:(dk + 1) * 128, :])
                    wr = pWv.tile([128, THDC], F32R, tag="wvrow", name=f"wv{dk}")
                    nc.vector.tensor_copy(out=wr, in_=raw)
                    wvrow.append(wr)
                for st in range(NT):
                    psv = pPproj.tile([128, THDC], F32, tag="psq", name=f"psv{st}")
                    for i in range(DK):
                        dk = (i + st) % DK
                        nc.tensor.matmul(psv, lhsT=xT[dk][:, st * 128:(st + 1) * 128],
                                         rhs=wvrow[dk], start=(i == 0), stop=(i == DK - 1))
                    va = vaug[st]
                    vav = va[:, :].rearrange("p (h c) -> p h c", h=HPC)
                    nc.vector.tensor_copy(out=vav[:, :, 0:Hd],
                                          in_=psv[:, :].rearrange("p (h c) -> p h c", h=HPC))
                    for h in range(HPC):
                        nc.vector.tensor_copy(out=va[:, h * 65 + 64:h * 65 + 65], in_=ones1)

                # K then Q projections + rope. K runs head-tile-major; Q
                # runs chunk-major so attention chunk 0 unlocks after 1/4 of
                # the Q work (K and V must be complete either way).
                for which, (w_dram, dst) in enumerate(((wk_d, kt), (wq_d, qt))):
                    wt = {}
                    for tt_i in (0, 2, 1, 3):
                        for dk in range(DK):
                            raw = pWr.tile([128, 128], F32, tag="wraw",
                                           name=f"wr{which}{tt_i}{dk}")
                            dma(out=raw, in_=w_dram[dk * 128:(dk + 1) * 128,
                                                    tt_i * 128:(tt_i + 1) * 128])
                            wr = pW.tile([128, 128], F32R, tag="wtile",
                                         name=f"w{which}{tt_i}{dk}")
                            nc.vector.tensor_copy(out=wr, in_=raw)
                            wt[(tt_i, dk)] = wr
                    if which == 0:
                        order = [(t, c) for t in (0, 2, 1, 3) for c in range(NCH)]
                    else:
                        order = [(t, c) for c in range(NCH) for t in (0, 2, 1, 3)]
                    for tt_i, c in order:
                        psq = pPproj.tile([128, 512], F32, tag="psq",
                                          name=f"psq{which}{tt_i}_{c}")
                        for i in range(DK):
                            dk = (i + tt_i * 2 + c) % DK
                            nc.tensor.matmul(
                                psq, lhsT=wt[(tt_i, dk)],
                                rhs=xT[dk][:, c * 512:(c + 1) * 512],
                                start=(i == 0), stop=(i == DK - 1))
                        rope(dst, tt_i, c, psq)

            # ---- phase 0: lambdas, gn scale/bias prep ----
            lamsb = pA.tile([1, 20], F32, tag="lamsb", name="lamsb")
            dma(out=lamsb, in_=lam_d[:, :].rearrange("a b -> (a b)").unsqueeze(0))
            lt = pA.tile([1, 8], F32, tag="lt", name="lt")
            nc.vector.tensor_tensor(out=lt[:, 0:4], in0=lamsb[:, 0:4], in1=lamsb[:, 4:8], op=MULT)
            nc.vector.tensor_tensor(out=lt[:, 4:8], in0=lamsb[:, 8:12], in1=lamsb[:, 12:16], op=MULT)
            nc.scalar.activation(out=lt, in_=lt, func=AF.Exp)
            lamv = pA.tile([1, 4], F32, tag="lamv", name="lamv")
            nc.vector.tensor_tensor(out=lamv, in0=lt[:, 0:4], in1=lt[:, 4:8], op=SUB)
            nc.vector.tensor_tensor(out=lamv, in0=lamv, in1=lamsb[:, 16:20], op=ADD)
            slam = pA.tile([1, 1], F32, tag="slam", name="slam")
            nc.scalar.activation(out=slam, in_=lamsb[:, 16:17], func=AF.Identity,
                                 scale=-1.0, bias=1.0)
            dscr = pD.tile([1, 8], F32, name="dscr")
            dma(out=dscr[0:1, 0:4], in_=lamv)
            dma(out=dscr[0:1, 4:5], in_=slam)
            lam_b = pA.tile([128, 4], F32, tag="lam_b", name="lam_b")
            dma(out=lam_b, in_=dscr[0:1, 0:4].squeeze(0).partition_broadcast(128))
            slam_b = pA.tile([128, 1], F32, tag="slam_b", name="slam_b")
            dma(out=slam_b, in_=dscr[0:1, 4:5].squeeze(0).partition_broadcast(128))
            gnw_eff = pA.tile([128, THDC], F32, tag="gnw_eff", name="gnw_eff")
            dma(out=gnw_eff, in_=gnw_d.ap().partition_broadcast(128))
            nc.vector.tensor_scalar_mul(gnw_eff, gnw_eff, slam_b[:, 0:1])
            gnb_eff = pA.tile([128, THDC], F32, tag="gnb_eff", name="gnb_eff")
            dma(out=gnb_eff, in_=gnb_d.ap().partition_broadcast(128))
            nc.vector.tensor_scalar_mul(gnb_eff, gnb_eff, slam_b[:, 0:1])
            # ---- phases 3-5: attention + per-head group norm + o_proj ----
            with tc.tile_pool(name="owyt", bufs=1) as pC0, \
                 tc.tile_pool(name="etile", bufs=6) as pE, \
                 tc.tile_pool(name="osb", bufs=4) as pO, \
                 tc.tile_pool(name="owh", bufs=3) as pOw, \
                 tc.tile_pool(name="gn", bufs=2) as pG, \
                 tc.tile_pool(name="outsb", bufs=3) as pOut, \
                 tc.tile_pool(name="wosb", bufs=1) as pWo, \
                 tc.tile_pool(name="ps_s", bufs=3, space="PSUM") as pPs, \
                 tc.tile_pool(name="ps_o", bufs=2, space="PSUM") as pPo, \
                 tc.tile_pool(name="ps_out", bufs=1, space="PSUM") as pPout, \
                 tc.tile_pool(name="ps_t2", bufs=2, space="PSUM") as pPt2:
                yt = [pC0.tile([128, S], F32R, tag=f"yt{i}", name=f"yt{i}")
                      for i in range(2)]
                lam_bh = []
                for h in range(HPC):
                    lb = pC0.tile([128, 4], F32, tag=f"lam_bh{h}", name=f"lam_bh{h}")
                    nc.vector.tensor_copy(out=lb,
                                          in_=lam_b[:, h:h + 1].broadcast_to((128, 4)))
                    lam_bh.append(lb)
                wo_sb = []
                for ci in range(2):
                    raw = pWo.tile([128, D], F32, tag="woraw", name=f"woraw{ci}")
                    dma(out=raw, in_=wo_d[ci * 128:(ci + 1) * 128, :])
                    wr = pWo.tile([128, D], F32R, tag=f"wo{ci}", name=f"wo{ci}")
                    nc.vector.tensor_copy(out=wr, in_=raw)
                    wo_sb.append(wr)

                for c in range(NCH):
                    cs = slice(c * 512, (c + 1) * 512)
                    for h in range(HPC):
                        tt1 = h // 2
                        tt2 = 2 + h // 2
                        ro = (h % 2) * 64
                        pso = [pPo.tile([65, 512], F32, tag="pso", name=f"pso{c}{h}{br}")
                               for br in range(2)]
                        for sk in range(NT):
                            va = vaug[sk][:, h * 65:(h + 1) * 65]
                            for br, tt_i in ((0, tt1), (1, tt2)):
                                pss = pPs.tile([128, 512], F32, tag="pss",
                                               name=f"pss{c}{h}{sk}{br}")
                                nc.tensor.matmul(pss,
                                                 lhsT=kt[tt_i][ro:ro + 64, sk * 128:(sk + 1) * 128],
                                                 rhs=qt[tt_i][ro:ro + 64, cs],
                                                 start=True, stop=True)
                                e = pE.tile([128, 512], F32R, tag="e", name=f"e{c}{h}{sk}{br}")
                                nc.scalar.activation(out=e, in_=pss, func=AF.Exp,
                                                     scale=QK_SCALE)
                                nc.tensor.matmul(pso[br], lhsT=va, rhs=e,
                                                 start=(sk == 0), stop=(sk == NT - 1))
                        owh = []
                        for br in range(2):
                            osb = pO.tile([65, 512], F32, tag="osb", name=f"osb{c}{h}{br}")
                            nc.vector.tensor_copy(out=osb, in_=pso[br])
                            owt = pOw.tile([128, HPC * (Hd + 1)], F32, tag=f"owh{br}",
                                           name=f"owh{c}{h}{br}")
                            for stl in range(4):
                                ptr = pPt2.tile([128, 65], F32, tag="ptr2",
                                                name=f"po{c}{h}{br}{stl}")
                                nc.tensor.transpose(ptr, osb[:, stl * 128:(stl + 1) * 128],
                                                    ident[0:65, 0:65])
                                nc.vector.tensor_copy(out=owt[:, stl * 65:(stl + 1) * 65],
                                                      in_=ptr)
                            owh.append(owt)

                        # group norm for this head over the chunk's 4 s-tiles
                        # (the 4 "groups" in the [128, 4, 65] views are s-tiles)
                        ow1, ow2 = owh
                        rec = pG.tile([128, 4], F32, tag="rec", name=f"rec{c}{h}")
                        nc.vector.reciprocal(rec, _rview(ow2))
                        rho = pG.tile([128, 4], F32, tag="rho", name=f"rho{c}{h}")
                        nc.vector.tensor_tensor(out=rho, in0=_rview(ow1), in1=rec, op=MULT)
                        nc.vector.tensor_tensor(out=rho, in0=rho, in1=lam_bh[h], op=MULT)
                        dt_ = pG.tile([128, THDC], F32, tag="dt", name=f"dt{c}{h}")
                        dtv = dt_[:, :].rearrange("p (g d) -> p g d", g=4)
                        nc.vector.tensor_tensor(out=dtv, in0=_hview(ow2), in1=_b64(rho), op=MULT)
                        nc.vector.tensor_tensor(out=dtv, in0=_hview(ow1), in1=dtv, op=SUB)
                        s1 = pG.tile([128, 4], F32, tag="s1", name=f"s1{c}{h}")
                        nc.vector.reduce_sum(out=s1, in_=dtv, axis=AX)
                        nc.vector.tensor_scalar_mul(s1, s1, -1.0 / Hd)
                        nc.vector.tensor_tensor(out=dtv, in0=dtv, in1=_b64(s1), op=ADD)
                        d2 = pG.tile([128, THDC], F32, tag="d2", name=f"d2{c}{h}")
                        nc.gpsimd.tensor_tensor(out=d2, in0=dt_, in1=dt_, op=MULT)
                        s2 = pG.tile([128, 4], F32, tag="s2", name=f"s2{c}{h}")
                        nc.vector.reduce_sum(out=s2,
                                             in_=d2[:, :].rearrange("p (g d) -> p g d", g=4),
                                             axis=AX)
                        nc.vector.tensor_tensor(out=rec, in0=_rview(ow1), in1=_rview(ow1), op=MULT)
                        nc.vector.tensor_scalar_mul(rec, rec, GN_EPS)
                        nc.vector.tensor_scalar_mul(s2, s2, 1.0 / Hd)
                        nc.vector.tensor_tensor(out=s2, in0=s2, in1=rec, op=ADD)
                        nc.scalar.activation(out=s2, in_=s2, func=AF.Sqrt)
                        nc.vector.reciprocal(s2, s2)
                        nc.vector.tensor_tensor(out=dtv, in0=dtv, in1=_b64(s2), op=MULT)
                        gw = gnw_eff[:, h * Hd:(h + 1) * Hd].unsqueeze(1) \
                            .broadcast_to((128, 4, Hd))
                        gb = gnb_eff[:, h * Hd:(h + 1) * Hd].unsqueeze(1) \
                            .broadcast_to((128, 4, Hd))
                        nc.gpsimd.tensor_tensor(out=dtv, in0=dtv, in1=gw, op=MULT)
                        nc.gpsimd.tensor_tensor(out=dtv, in0=dtv, in1=gb, op=ADD)
                        for stl in range(4):
                            st = c * 4 + stl
                            ptr = pPt2.tile([64, 128], F32, tag="ptr2", name=f"py{c}{h}{stl}")
                            nc.tensor.transpose(ptr, dt_[:, stl * Hd:(stl + 1) * Hd], ident)
                            nc.vector.tensor_copy(
                                out=yt[h // 2][ro:ro + 64, st * 128:(st + 1) * 128],
                                in_=ptr)

                    # o_proj for this chunk's 4 s-tiles
                    for stl in range(4):
                        st = c * 4 + stl
                        for oc in range(2):
                            pout = pPout.tile([128, 512], F32, tag="pout", name=f"pout{st}_{oc}")
                            for ci in range(2):
                                nc.tensor.matmul(pout,
                                                 lhsT=yt[ci][:, st * 128:(st + 1) * 128],
                                                 rhs=wo_sb[ci][:, oc * 512:(oc + 1) * 512],
                                                 start=(ci == 0), stop=(ci == 1))
                            ost = pOut.tile([128, 512], F32, tag="ost", name=f"ost{st}_{oc}")
                            nc.vector.tensor_copy(out=ost, in_=pout)
                            dma(out=y_d[st * 128:(st + 1) * 128, oc * 512:(oc + 1) * 512],
                                in_=ost)

    _split_multi_waits(nc)
    return nc


_CACHE = {}


def _get_module():
    if "nc" not in _CACHE:
        _CACHE["nc"] = build_module()
        _CACHE["tables"] = _rope_tables()
    return _CACHE["nc"], _CACHE["tables"]


def kernel(x, Wq, Wk, Wv, Wo, lambda_q1, lambda_k1, lambda_q2, lambda_k2,
           lambda_init, gn_weight, gn_bias):
    from concourse.bass_utils import run_bass_kernel_spmd

    x = np.ascontiguousarray(np.asarray(x, dtype=np.float32))
    Wq = np.asarray(Wq, dtype=np.float32)
    Wk = np.asarray(Wk, dtype=np.float32)
    Wv = np.asarray(Wv, dtype=np.float32)
    Wo = np.asarray(Wo, dtype=np.float32)
    lq1 = np.asarray(lambda_q1, dtype=np.float32)
    lk1 = np.asarray(lambda_k1, dtype=np.float32)
    lq2 = np.asarray(lambda_q2, dtype=np.float32)
    lk2 = np.asarray(lambda_k2, dtype=np.float32)
    lam_init = np.float32(np.asarray(lambda_init).reshape(()))
    gnw = np.asarray(gn_weight, dtype=np.float32)
    gnb = np.asarray(gn_bias, dtype=np.float32)

    nc, (cosT, ssinT) = _get_module()

    in_maps = []
    for core in range(8):
        b = core // 4
        hb = (core % 4) * HPC
        c1 = slice(hb * Hd, (hb + HPC) * Hd)
        c2 = slice(H * Hd + hb * Hd, H * Hd + (hb + HPC) * Hd)
        lam = np.stack([lq1[hb:hb + HPC], lk1[hb:hb + HPC],
                        lq2[hb:hb + HPC], lk2[hb:hb + HPC],
                        np.full(HPC, lam_init, np.float32)]).astype(np.float32)
        in_maps.append({
            "x": np.ascontiguousarray(x[b]),
            "wq": np.ascontiguousarray(np.concatenate([Wq[:, c1], Wq[:, c2]], axis=1)),
            "wk": np.ascontiguousarray(np.concatenate([Wk[:, c1], Wk[:, c2]], axis=1)),
            "wv": np.ascontiguousarray(Wv[:, c1]),
            "wo": np.ascontiguousarray(Wo[c1, :]),
            "lam": lam,
            "gnw": np.ascontiguousarray(gnw[c1]),
            "gnb": np.ascontiguousarray(gnb[c1]),
            "cosT": cosT,
            "ssinT": ssinT,
        })

    last_err = None
    for attempt in range(3):
        try:
            res = run_bass_kernel_spmd(nc, in_maps, core_ids=list(range(8)))
            break
        except Exception as e:  # transient axon/device hiccups
            last_err = e
            time.sleep(10 * (attempt + 1))
    else:
        raise last_err

    out = np.zeros((B, S, D), dtype=np.float32)
    for core in range(8):
        out[core // 4] += res.results[core]["y"]
    return out



# revision 9
# speedup vs baseline: 1.1348x; 1.1348x over previous
"""Differential attention (nn_DifferentialAttention_84679575208071) on 8 TRN2
NeuronCores via Bass/Tile.

Sharding: hybrid data-parallel x tensor-parallel. Core c handles batch c//4 and
heads 4*(c%4) .. 4*(c%4)+4 (B=2, H=16 -> 2 batch groups x 4-way head split).
Each core computes its 4 heads' attention + group-norm + partial o_proj; the
host sums the 4 partial (S, D) outputs per batch and stacks the 2 batches.

Per-core pipeline (all matmuls bf16 = 1 col/cycle on the PE):
  xT        via DMA-transpose (xbar) straight from DRAM -> SBUF, no PE work
  qT/kT     = (W slice)^T x^T, rope applied with the rotate_half PE perm fed
              from a (q*sin) product so no PSUM->SBUF stage copy is needed
  v         = x Wv (row layout) + a ones column per head so the PV matmul
              also yields softmax denominators
  attention chunk-major, per (chunk, head, branch): 16 score matmuls
              [128kpos x 512q] -> exp on Act (bf16 out) -> PV in the FLIPPED
              orientation out[q, ch]: lhsT = e-subtile, rhs = v-aug; this
              halves PV streaming cost and lands the output already in
              [s-partition, channel] layout for group-norm (no transposes)
  diff      = O1 - (lam*r1/r2) * O2 folded through group-norm (eps scaled by
              r1^2), then gn_weight*(1-lam_init) + bias, o_proj per chunk
  scheduling: projection / o_proj / output-transpose work is interleaved into
              the attention stream via a task queue pulled once per score tile
              so the PE never idles while the Act engine streams exps.
"""
import json
import time
from collections import deque

import numpy as np
import ml_dtypes

import bass_rust
import concourse.bass as bass
import concourse.mybir as mybir
import concourse.tile as tile

F32 = mybir.dt.float32
BF16 = mybir.dt.bfloat16
MULT = mybir.AluOpType.mult
ADD = mybir.AluOpType.add
SUB = mybir.AluOpType.subtract
AX = mybir.AxisListType.X
AF = mybir.ActivationFunctionType

B, S, D = 2, 2048, 1024
H, Hd = 16, 64
HPC = 4               # heads per core
THDC = HPC * Hd       # 256 channels per core
NT = S // 128         # 16 s-tiles
NCH = S // 512        # 4 chunks
DK = D // 128         # 8 contraction tiles
LAMBDA_INIT = 0.8
GN_EPS = 1e-5
ROPE_BASE = 10000.0
QK_SCALE = float(Hd) ** -0.5  # 0.125

NPBF16 = ml_dtypes.bfloat16


def _split_multi_waits(nc):
    """This container's walrus rejects >1 sync wait per instruction. Hoist
    extra waits onto same-engine NoOps inserted right before the instruction
    (engine queues are in-order, so semantics are unchanged)."""
    d = json.loads(bass_rust.module_to_json_string(nc.m))
    ctr = 0
    for f in d["functions"]:
        for bb in f["blocks"]:
            out = []
            for inst in bb.get("instructions", []):
                si = inst.get("sync_info")
                waits = si.get("on_wait", []) if si else []
                if len(waits) > 1:
                    for w in waits[:-1]:
                        ctr += 1
                        out.append({
                            "debug": inst.get("debug", 0),
                            "engine": inst["engine"],
                            "ins": [],
                            "name": f"WSPLIT-{ctr}",
                            "opcode": "NoOp",
                            "outs": [],
                            "sync_info": {"on_update": [], "on_wait": [w]},
                        })
                    si["on_wait"] = [waits[-1]]
                out.append(inst)
            bb["instructions"] = out
    nc.m = bass_rust.module_from_json_string(json.dumps(d))


def _rope_tables():
    inv_freq = 1.0 / (ROPE_BASE ** (np.arange(0, Hd, 2, dtype=np.float32) / Hd))
    t = np.arange(S, dtype=np.float32)
    freqs = np.outer(t, inv_freq).astype(np.float32)
    emb = np.concatenate([freqs, freqs], axis=-1)       # (S, Hd)
    cos = np.cos(emb).astype(NPBF16)
    sin = np.sin(emb).astype(NPBF16)
    return np.ascontiguousarray(cos.T), np.ascontiguousarray(sin.T)  # (Hd, S)


def _b64(ap):
    """[128, 4] -> [128, 4, 64] broadcast along a new inner dim."""
    return ap.unsqueeze(2).broadcast_to((128, 4, Hd))


def build_module():
    nc = bass.Bass(trn_type="TRN2")

    x_d = nc.dram_tensor("x", [S, D], BF16, kind="ExternalInput")
    wq_d = nc.dram_tensor("wq", [D, 512], BF16, kind="ExternalInput")
    wk_d = nc.dram_tensor("wk", [D, 512], BF16, kind="ExternalInput")
    wv_d = nc.dram_tensor("wv", [D, THDC], BF16, kind="ExternalInput")
    wo_d = nc.dram_tensor("wo", [THDC, D], BF16, kind="ExternalInput")
    lam_d = nc.dram_tensor("lam", [5, HPC], F32, kind="ExternalInput")
    gnw_d = nc.dram_tensor("gnw", [THDC], F32, kind="ExternalInput")
    gnb_d = nc.dram_tensor("gnb", [THDC], F32, kind="ExternalInput")
    cos_d = nc.dram_tensor("cosT", [Hd, S], BF16, kind="ExternalInput")
    sin_d = nc.dram_tensor("ssinT", [Hd, S], BF16, kind="ExternalInput")
    y_d = nc.dram_tensor("y", [S, D], F32, kind="ExternalOutput")

    ident_d = nc.inline_tensor(np.eye(128, dtype=np.float32).astype(NPBF16),
                               name="identity")
    # rotate_half as a 128x128 +-1 permutation (block-diag per 64-row head):
    # perm @ t gives rh(t); transposed for the lhsT slot
    _m64 = np.zeros((64, 64), dtype=np.float32)
    for _dd in range(32):
        _m64[_dd, _dd + 32] = -1.0
        _m64[_dd + 32, _dd] = 1.0
    _m128 = np.zeros((128, 128), dtype=np.float32)
    _m128[:64, :64] = _m64
    _m128[64:, 64:] = _m64
    rotm_d = nc.inline_tensor(np.ascontiguousarray(_m128.T).astype(NPBF16),
                              name="rotm")

    dma = nc.sync.dma_start      # HWDGE
    dmaT = nc.sync.dma_start_transpose

    with tile.TileContext(nc) as tc:
        with tc.tile_pool(name="persist", bufs=1) as pA, \
             tc.tile_pool(name="dscr", bufs=1, space="DRAM") as pD, \
             tc.tile_pool(name="tsin", bufs=2) as pT, \
             tc.tile_pool(name="e", bufs=4) as pE, \
             tc.tile_pool(name="ow", bufs=4) as pOw, \
             tc.tile_pool(name="gn", bufs=2) as pG, \
             tc.tile_pool(name="dtb", bufs=2) as pDt, \
             tc.tile_pool(name="ost", bufs=3) as pOst, \
             tc.tile_pool(name="ps_pv", bufs=2, space="PSUM") as pPv, \
             tc.tile_pool(name="ps_q", bufs=1, space="PSUM") as pPq, \
             tc.tile_pool(name="ps_perm", bufs=1, space="PSUM") as pPerm, \
             tc.tile_pool(name="ps_py", bufs=1, space="PSUM") as pPy, \
             tc.tile_pool(name="ps_v", bufs=1, space="PSUM") as pVout:

            # ---- persistent SBUF ----
            xT = [pA.tile([128, S], BF16, tag=f"xT{i}", name=f"xT{i}")
                  for i in range(DK)]
            qt = [pA.tile([128, S], BF16, tag=f"qt{i}", name=f"qt{i}") for i in range(4)]
            kt = [pA.tile([128, S], BF16, tag=f"kt{i}", name=f"kt{i}") for i in range(4)]
            vaug = [pA.tile([128, HPC * (Hd + 1)], BF16, tag=f"va{i}", name=f"va{i}")
                    for i in range(NT)]
            yt = [pA.tile([128, S], BF16, tag=f"yt{i}", name=f"yt{i}") for i in range(2)]
            cos2h = pA.tile([128, S], BF16, tag="cos2h", name="cos2h")
            sin2h = pA.tile([128, S], BF16, tag="sin2h", name="sin2h")
            wq_sb = [pA.tile([128, 512], BF16, tag=f"wq{i}", name=f"wq{i}")
                     for i in range(DK)]
            wk_sb = [pA.tile([128, 512], BF16, tag=f"wk{i}", name=f"wk{i}")
                     for i in range(DK)]
            wv_sb = [pA.tile([128, THDC], BF16, tag=f"wv{i}", name=f"wv{i}")
                     for i in range(DK)]
            wo_sb = [pA.tile([128, D], BF16, tag=f"wo{i}", name=f"wo{i}")
                     for i in range(2)]
            ident = pA.tile([128, 128], BF16, tag="ident", name="ident")
            rotm = pA.tile([128, 128], BF16, tag="rotm", name="rotm")
            ones4 = pA.tile([128, 4], BF16, tag="ones4", name="ones4")

            # ---- DMAs (order matters for prelude latency) ----
            dma(out=ident, in_=ident_d.ap())
            dma(out=rotm, in_=rotm_d.ap())
            nc.vector.memset(ones4, 1.0)
            for dk in range(DK):
                dma(out=wv_sb[dk], in_=wv_d[dk * 128:(dk + 1) * 128, :])
            for dk in range(DK):
                dma(out=wk_sb[dk], in_=wk_d[dk * 128:(dk + 1) * 128, :])
            dma(out=cos2h[0:64, :], in_=cos_d[:, :])
            dma(out=cos2h[64:128, :], in_=cos_d[:, :])
            dma(out=sin2h[0:64, :], in_=sin_d[:, :])
            dma(out=sin2h[64:128, :], in_=sin_d[:, :])
            # x^T via xbar DMA transpose, chunk-major so chunk 0 lands first
            for c in range(NCH):
                cs = slice(c * 512, (c + 1) * 512)
                for dk in range(DK):
                    dmaT(out=xT[dk][:, cs],
                         in_=x_d[cs, dk * 128:(dk + 1) * 128])
                if c == 0:
                    for dk in range(DK):
                        dma(out=wq_sb[dk], in_=wq_d[dk * 128:(dk + 1) * 128, :])
            for ci in range(2):
                dma(out=wo_sb[ci], in_=wo_d[ci * 128:(ci + 1) * 128, :])

            # ---- lambdas, gn scale/bias prep ----
            lamsb = pA.tile([1, 20], F32, tag="lamsb", name="lamsb")
            dma(out=lamsb, in_=lam_d[:, :].rearrange("a b -> (a b)").unsqueeze(0))
            lt = pA.tile([1, 8], F32, tag="lt", name="lt")
            nc.vector.tensor_tensor(out=lt[:, 0:4], in0=lamsb[:, 0:4],
                                    in1=lamsb[:, 4:8], op=MULT)
            nc.vector.tensor_tensor(out=lt[:, 4:8], in0=lamsb[:, 8:12],
                                    in1=lamsb[:, 12:16], op=MULT)
            nc.scalar.activation(out=lt, in_=lt, func=AF.Exp)
            lamv = pA.tile([1, 4], F32, tag="lamv", name="lamv")
            nc.vector.tensor_tensor(out=lamv, in0=lt[:, 0:4], in1=lt[:, 4:8], op=SUB)
            nc.vector.tensor_tensor(out=lamv, in0=lamv, in1=lamsb[:, 16:20], op=ADD)
            slam = pA.tile([1, 1], F32, tag="slam", name="slam")
            nc.scalar.activation(out=slam, in_=lamsb[:, 16:17], func=AF.Identity,
                                 scale=-1.0, bias=1.0)
            dscr = pD.tile([1, 8], F32, name="dscr")
            dma(out=dscr[0:1, 0:4], in_=lamv)
            dma(out=dscr[0:1, 4:5], in_=slam)
            lam_b = pA.tile([128, 4], F32, tag="lam_b", name="lam_b")
            dma(out=lam_b, in_=dscr[0:1, 0:4].squeeze(0).partition_broadcast(128))
            slam_b = pA.tile([128, 1], F32, tag="slam_b", name="slam_b")
            dma(out=slam_b, in_=dscr[0:1, 4:5].squeeze(0).partition_broadcast(128))
            gnw_eff = pA.tile([128, THDC], F32, tag="gnw_eff", name="gnw_eff")
            dma(out=gnw_eff, in_=gnw_d.ap().partition_broadcast(128))
            nc.vector.tensor_scalar_mul(gnw_eff, gnw_eff, slam_b[:, 0:1])
            gnb_eff = pA.tile([128, THDC], F32, tag="gnb_eff", name="gnb_eff")
            dma(out=gnb_eff, in_=gnb_d.ap().partition_broadcast(128))
            nc.vector.tensor_scalar_mul(gnb_eff, gnb_eff, slam_b[:, 0:1])
            lam_bh = []
            for h in range(HPC):
                lb = pA.tile([128, 4], F32, tag=f"lam_bh{h}", name=f"lam_bh{h}")
                nc.vector.tensor_copy(out=lb,
                                      in_=lam_b[:, h:h + 1].broadcast_to((128, 4)))
                lam_bh.append(lb)

            # ---- task generators ----
            def g_proj(w_sb, dst, tt, c, idx):
                cs = slice(c * 512, (c + 1) * 512)
                psq = pPq.tile([128, 512], F32, tag="psq", name=f"psq{idx}")
                for i in range(DK):
                    dk = (i + tt * 2 + c) % DK
                    nc.tensor.matmul(psq, lhsT=w_sb[dk][:, tt * 128:(tt + 1) * 128],
                                     rhs=xT[dk][:, cs],
                                     start=(i == 0), stop=(i == DK - 1))
                    yield
                tsin = pT.tile([128, 512], BF16, tag="tsin", name=f"ts{idx}")
                nc.vector.tensor_tensor(out=tsin, in0=psq, in1=sin2h[:, cs], op=MULT)
                perm = pPerm.tile([128, 512], F32, tag="perm", name=f"pm{idx}")
                nc.tensor.matmul(perm, lhsT=rotm, rhs=tsin, start=True, stop=True)
                yield
                nc.vector.tensor_tensor(out=dst[tt][:, cs], in0=psq,
                                        in1=cos2h[:, cs], op=MULT)
                nc.vector.tensor_tensor(out=dst[tt][:, cs], in0=dst[tt][:, cs],
                                        in1=perm, op=ADD)

            def g_vproj(st):
                # single full-region tile per group: the whole-bank pending-zero
                # of start=True makes sub-bank double-buffering unsafe, so rely
                # on the WAR dep against the previous group's copy instead
                psv = pVout.tile([128, 512], F32, tag="vout", name=f"psv{st}")[:, 0:THDC]
                for i in range(DK):
                    dk = (i + st) % DK
                    nc.tensor.matmul(psv, lhsT=xT[dk][:, st * 128:(st + 1) * 128],
                                     rhs=wv_sb[dk], start=(i == 0), stop=(i == DK - 1))
                    yield
                va = vaug[st]
                vav = va[:, :].rearrange("p (h c) -> p h c", h=HPC)
                nc.vector.tensor_copy(
                    out=vav[:, :, 0:Hd],
                    in_=psv.rearrange("p (h c) -> p h c", h=HPC))
                rv = vav[:, :, Hd:Hd + 1].rearrange("p h c -> p (h c)")
                nc.vector.tensor_copy(out=rv, in_=ones4)

            def g_py(c, h, dtb):
                ro = (h % 2) * 64
                for stl in range(4):
                    st = c * 4 + stl
                    ptr = pPy.tile([64, 128], BF16, tag="py", name=f"py{c}{h}{stl}")
                    nc.tensor.transpose(ptr, dtb[:, stl * Hd:(stl + 1) * Hd], ident)
                    yield
                    nc.vector.tensor_copy(
                        out=yt[h // 2][ro:ro + 64, st * 128:(st + 1) * 128], in_=ptr)

            def g_oproj(c, st, oc, pool, tag):
                pout = pool.tile([128, 512], F32, tag=tag, name=f"po{st}_{oc}")
                for ci in range(2):
                    nc.tensor.matmul(pout, lhsT=yt[ci][:, st * 128:(st + 1) * 128],
                                     rhs=wo_sb[ci][:, oc * 512:(oc + 1) * 512],
                                     start=(ci == 0), stop=(ci == 1))
                    yield
                ost = pOst.tile([128, 512], F32, tag="ost", name=f"ost{st}_{oc}")
                nc.vector.tensor_copy(out=ost, in_=pout)
                dma(out=y_d[st * 128:(st + 1) * 128, oc * 512:(oc + 1) * 512],
                    in_=ost)

            # two-priority task queues: prim (py/o_proj, pulled first),
            # sec (K/Q proj with ordering markers)
            prim = deque()
            sec = deque()

            def drain_until(key):
                # run everything in prim+sec up to (and incl.) marker `key`
                while True:
                    q = prim if prim else sec
                    if not q:
                        return
                    obj = q[0]
                    if isinstance(obj, str):
                        sec.popleft()
                        if obj == key:
                            return
                        continue
                    try:
                        next(obj)
                    except StopIteration:
                        q.popleft()

            def drain_all():
                while prim or sec:
                    q = prim if prim else sec
                    obj = q[0]
                    if isinstance(obj, str):
                        q.popleft()
                        continue
                    try:
                        next(obj)
                    except StopIteration:
                        q.popleft()

            def pull2(n=1):
                emitted = 0
                while emitted < n:
                    if prim:
                        q = prim
                    elif sec and not isinstance(sec[0], str):
                        q = sec
                    else:
                        return
                    try:
                        next(q[0])
                        emitted += 1
                    except StopIteration:
                        q.popleft()

            # ---- prelude: V proj + K(tt0/tt2) + Q(c0, tt0/tt2), interleaved
            # at group granularity so rope DVE reads overlap the next group ----
            pre = deque()
            for st in range(NT):
                pre.append(g_vproj(st))
            kq_pre = deque()
            gi = 0
            for c in range(NCH):
                kq_pre.append(g_proj(wk_sb, kt, 0, c, f"k0{c}"))
            for c in range(NCH):
                kq_pre.append(g_proj(wk_sb, kt, 2, c, f"k2{c}"))
            kq_pre.append(g_proj(wq_sb, qt, 0, 0, "q00"))
            kq_pre.append(g_proj(wq_sb, qt, 2, 0, "q20"))
            # round-robin merge: one V group then one K/Q group
            while pre or kq_pre:
                if pre:
                    for _ in pre.popleft():
                        pass
                if kq_pre:
                    for _ in kq_pre.popleft():
                        pass

            # ---- secondary queue: rest of K and Q with markers ----
            for c in range(NCH):
                sec.append(g_proj(wk_sb, kt, 1, c, f"k1{c}"))
            for c in range(NCH):
                sec.append(g_proj(wk_sb, kt, 3, c, f"k3{c}"))
            sec.append(g_proj(wq_sb, qt, 1, 0, "q10"))
            sec.append(g_proj(wq_sb, qt, 3, 0, "q30"))
            sec.append("c0h2")
            for c in range(1, NCH):
                sec.append(g_proj(wq_sb, qt, 0, c, f"q0{c}"))
                sec.append(g_proj(wq_sb, qt, 2, c, f"q2{c}"))
                sec.append(f"c{c}h0")
                sec.append(g_proj(wq_sb, qt, 1, c, f"q1{c}"))
                sec.append(g_proj(wq_sb, qt, 3, c, f"q3{c}"))
                sec.append(f"c{c}h2")

            # ---- attention + group norm ----
            with tc.tile_pool(name="ps_s", bufs=2, space="PSUM") as pPs:
                for c in range(NCH):
                    cs = slice(c * 512, (c + 1) * 512)
                    for h in range(HPC):
                        if h == 0 and c > 0:
                            drain_until(f"c{c}h0")
                        if h == 2:
                            drain_until(f"c{c}h2")
                        ro = (h % 2) * 64
                        ows = []
                        for br, tt in ((0, h // 2), (1, 2 + h // 2)):
                            pv = pPv.tile([128, 4, 65], F32, tag="pv",
                                          name=f"pv{c}{h}{br}")
                            prev_e = None
                            for sk in range(NT):
                                pss = pPs.tile([128, 512], F32, tag="pss",
                                               name=f"pss{c}{h}{br}{sk}")
                                nc.tensor.matmul(
                                    pss,
                                    lhsT=kt[tt][ro:ro + 64, sk * 128:(sk + 1) * 128],
                                    rhs=qt[tt][ro:ro + 64, cs],
                                    start=True, stop=True)
                                e = pE.tile([128, 512], BF16, tag="e",
                                            name=f"e{c}{h}{br}{sk}")
                                nc.scalar.activation(out=e, in_=pss, func=AF.Exp,
                                                     scale=QK_SCALE)
                                if prev_e is not None:
                                    va_h = vaug[sk - 1][:, h * 65:(h + 1) * 65]
                                    for qs in range(4):
                                        # start=True marks the whole PSUM bank
                                        # pending-zero, so only the first
                                        # matmul of the bank may set it
                                        nc.tensor.matmul(
                                            pv[:, qs, :],
                                            lhsT=prev_e[:, qs * 128:(qs + 1) * 128],
                                            rhs=va_h,
                                            start=(sk - 1 == 0 and qs == 0),
                                            stop=False)
                                prev_e = e
                                pull2(1)
                            va_h = vaug[NT - 1][:, h * 65:(h + 1) * 65]
                            for qs in range(4):
                                nc.tensor.matmul(
                                    pv[:, qs, :],
                                    lhsT=prev_e[:, qs * 128:(qs + 1) * 128],
                                    rhs=va_h, start=False, stop=True)
                            ow = pOw.tile([128, 4, 65], F32, tag="ow",
                                          name=f"ow{c}{h}{br}")
                            nc.vector.tensor_copy(out=ow, in_=pv)
                            ows.append(ow)

                        # group norm for this head (4 s-tiles = the 4 "groups")
                        ow1, ow2 = ows
                        r1 = ow1[:, :, Hd]
                        r2 = ow2[:, :, Hd]
                        rec = pG.tile([128, 4], F32, tag="rec", name=f"rec{c}{h}")
                        nc.vector.reciprocal(rec, r2)
                        rho = pG.tile([128, 4], F32, tag="rho", name=f"rho{c}{h}")
                        nc.vector.tensor_tensor(out=rho, in0=r1, in1=rec, op=MULT)
                        nc.vector.tensor_tensor(out=rho, in0=rho, in1=lam_bh[h], op=MULT)
                        dt_ = pG.tile([128, THDC], F32, tag="dt", name=f"dt{c}{h}")
                        dtv = dt_[:, :].rearrange("p (g d) -> p g d", g=4)
                        nc.vector.tensor_tensor(out=dtv, in0=ow2[:, :, 0:Hd],
                                                in1=_b64(rho), op=MULT)
                        nc.vector.tensor_tensor(out=dtv, in0=ow1[:, :, 0:Hd],
                                                in1=dtv, op=SUB)
                        s1 = pG.tile([128, 4], F32, tag="s1", name=f"s1{c}{h}")
                        nc.vector.reduce_sum(out=s1, in_=dtv, axis=AX)
                        nc.vector.tensor_scalar_mul(s1, s1, -1.0 / Hd)
                        nc.vector.tensor_tensor(out=dtv, in0=dtv, in1=_b64(s1), op=ADD)
                        d2 = pG.tile([128, THDC], F32, tag="d2", name=f"d2{c}{h}")
                        nc.gpsimd.tensor_tensor(out=d2, in0=dt_, in1=dt_, op=MULT)
                        s2 = pG.tile([128, 4], F32, tag="s2", name=f"s2{c}{h}")
                        nc.vector.reduce_sum(
                            out=s2, in_=d2[:, :].rearrange("p (g d) -> p g d", g=4),
                            axis=AX)
                        nc.vector.tensor_tensor(out=rec, in0=r1, in1=r1, op=MULT)
                        nc.vector.tensor_scalar_mul(rec, rec, GN_EPS)
                        nc.vector.tensor_scalar_mul(s2, s2, 1.0 / Hd)
                        nc.vector.tensor_tensor(out=s2, in0=s2, in1=rec, op=ADD)
                        nc.scalar.activation(out=s2, in_=s2, func=AF.Sqrt)
                        nc.vector.reciprocal(s2, s2)
                        nc.vector.tensor_tensor(out=dtv, in0=dtv, in1=_b64(s2), op=MULT)
                        gw = gnw_eff[:, h * Hd:(h + 1) * Hd].unsqueeze(1) \
                            .broadcast_to((128, 4, Hd))
                        gb = gnb_eff[:, h * Hd:(h + 1) * Hd].unsqueeze(1) \
                            .broadcast_to((128, 4, Hd))
                        dtb = pDt.tile([128, THDC], BF16, tag="dtb", name=f"dtb{c}{h}")
                        dtbv = dtb[:, :].rearrange("p (g d) -> p g d", g=4)
                        nc.gpsimd.tensor_tensor(out=dtbv, in0=dtv, in1=gw, op=MULT)
                        nc.gpsimd.tensor_tensor(out=dtbv, in0=dtbv, in1=gb, op=ADD)
                        prim.append(g_py(c, h, dtb))
                        if h == HPC - 1 and c < NCH - 1:
                            for stl in range(4):
                                st = c * 4 + stl
                                for oc in range(2):
                                    prim.append(g_oproj(c, st, oc, pVout, "vout"))
                drain_all()
            # pPs closed: 2 banks free for double-buffered tail o_proj
            with tc.tile_pool(name="ps_tail", bufs=2, space="PSUM") as pTail:
                tail = []
                for stl in range(4):
                    st = (NCH - 1) * 4 + stl
                    for oc in range(2):
                        tail.append(g_oproj(NCH - 1, st, oc, pTail, "tail"))
                # interleave pairs of groups so the two PSUM slots overlap
                for i in range(0, len(tail), 2):
                    alive = list(tail[i:i + 2])
                    while alive:
                        for g in list(alive):
                            try:
                                next(g)
                            except StopIteration:
                                alive.remove(g)

    _split_multi_waits(nc)
    return nc


_CACHE = {}


def _get_module():
    if "nc" not in _CACHE:
        _CACHE["nc"] = build_module()
        _CACHE["tables"] = _rope_tables()
    return _CACHE["nc"], _CACHE["tables"]


def kernel(x, Wq, Wk, Wv, Wo, lambda_q1, lambda_k1, lambda_q2, lambda_k2,
           lambda_init, gn_weight, gn_bias):
    from concourse.bass_utils import run_bass_kernel_spmd

    x = np.asarray(x, dtype=np.float32)
    Wq = np.asarray(Wq, dtype=np.float32)
    Wk = np.asarray(Wk, dtype=np.float32)
    Wv = np.asarray(Wv, dtype=np.float32)
    Wo = np.asarray(Wo, dtype=np.float32)
    lq1 = np.asarray(lambda_q1, dtype=np.float32)
    lk1 = np.asarray(lambda_k1, dtype=np.float32)
    lq2 = np.asarray(lambda_q2, dtype=np.float32)
    lk2 = np.asarray(lambda_k2, dtype=np.float32)
    lam_init = np.float32(np.asarray(lambda_init).reshape(()))
    gnw = np.asarray(gn_weight, dtype=np.float32)
    gnb = np.asarray(gn_bias, dtype=np.float32)

    nc, (cosT, ssinT) = _get_module()

    xb = [np.ascontiguousarray(x[b].astype(NPBF16)) for b in range(B)]
    in_maps = []
    for core in range(8):
        b = core // 4
        hb = (core % 4) * HPC
        c1 = slice(hb * Hd, (hb + HPC) * Hd)
        c2 = slice(H * Hd + hb * Hd, H * Hd + (hb + HPC) * Hd)
        lam = np.stack([lq1[hb:hb + HPC], lk1[hb:hb + HPC],
                        lq2[hb:hb + HPC], lk2[hb:hb + HPC],
                        np.full(HPC, lam_init, np.float32)]).astype(np.float32)
        in_maps.append({
            "x": xb[b],
            "wq": np.ascontiguousarray(
                np.concatenate([Wq[:, c1], Wq[:, c2]], axis=1).astype(NPBF16)),
            "wk": np.ascontiguousarray(
                np.concatenate([Wk[:, c1], Wk[:, c2]], axis=1).astype(NPBF16)),
            "wv": np.ascontiguousarray(Wv[:, c1].astype(NPBF16)),
            "wo": np.ascontiguousarray(Wo[c1, :].astype(NPBF16)),
            "lam": lam,
            "gnw": np.ascontiguousarray(gnw[c1]),
            "gnb": np.ascontiguousarray(gnb[c1]),
            "cosT": cosT,
            "ssinT": ssinT,
        })

    last_err = None
    for attempt in range(3):
        try:
            res = run_bass_kernel_spmd(nc, in_maps, core_ids=list(range(8)))
            break
        except Exception as e:  # transient axon/device hiccups
            last_err = e
            time.sleep(10 * (attempt + 1))
    else:
        raise last_err

    out = np.zeros((B, S, D), dtype=np.float32)
    for core in range(8):
        out[core // 4] += res.results[core]["y"]
    return out


# revision 25
# speedup vs baseline: 1.2586x; 1.1090x over previous
"""Differential attention (nn_DifferentialAttention_84679575208071) on 8 TRN2
NeuronCores via Bass/Tile.

Sharding: hybrid data-parallel x tensor-parallel. Core c handles batch c//4 and
heads 4*(c%4) .. 4*(c%4)+4 (B=2, H=16 -> 2 batch groups x 4-way head split).
Each core computes its 4 heads' attention + group-norm + partial o_proj; the
host sums the 4 partial (S, D) outputs per batch and stacks the 2 batches.

Per-core pipeline (all matmuls bf16 = 1 col/cycle on the PE):
  xT        via DMA-transpose (xbar) straight from DRAM -> SBUF, no PE work
  qT/kT     = (W slice)^T x^T, rope applied with the rotate_half PE perm fed
              from a (q*sin) product so no PSUM->SBUF stage copy is needed
  v         = x Wv (row layout) + a ones column per head so the PV matmul
              also yields softmax denominators
  attention chunk-major, per (chunk, head, branch): 16 score matmuls
              [128kpos x 512q] -> exp on Act (bf16 out) -> PV in the FLIPPED
              orientation out[q, ch]: lhsT = e-subtile, rhs = v-aug; this
              halves PV streaming cost and lands the output already in
              [s-partition, channel] layout for group-norm (no transposes)
  diff      = O1 - (lam*r1/r2) * O2 folded through group-norm (eps scaled by
              r1^2), then gn_weight*(1-lam_init) + bias, o_proj per chunk
  scheduling: projection / o_proj / output-transpose work is interleaved into
              the attention stream via a task queue pulled once per score tile
              so the PE never idles while the Act engine streams exps.
"""
import json
import time
from collections import deque

import numpy as np
import ml_dtypes

import bass_rust
import concourse.bass as bass
import concourse.mybir as mybir
import concourse.tile as tile

F32 = mybir.dt.float32
BF16 = mybir.dt.bfloat16
MULT = mybir.AluOpType.mult
ADD = mybir.AluOpType.add
SUB = mybir.AluOpType.subtract
AX = mybir.AxisListType.X
AF = mybir.ActivationFunctionType

B, S, D = 2, 2048, 1024
H, Hd = 16, 64
HPC = 4               # heads per core
THDC = HPC * Hd       # 256 channels per core
NT = S // 128         # 16 s-tiles
NCH = S // 512        # 4 chunks
DK = D // 128         # 8 contraction tiles
LAMBDA_INIT = 0.8
GN_EPS = 1e-5
ROPE_BASE = 10000.0
QK_SCALE = float(Hd) ** -0.5  # 0.125

NPBF16 = ml_dtypes.bfloat16


def _split_multi_waits(nc):
    """This container's walrus rejects >1 sync wait per instruction. Hoist
    extra waits onto same-engine NoOps inserted right before the instruction
    (engine queues are in-order, so semantics are unchanged)."""
    d = json.loads(bass_rust.module_to_json_string(nc.m))
    ctr = 0
    for f in d["functions"]:
        for bb in f["blocks"]:
            out = []
            for inst in bb.get("instructions", []):
                si = inst.get("sync_info")
                waits = si.get("on_wait", []) if si else []
                if len(waits) > 1:
                    for w in waits[:-1]:
                        ctr += 1
                        out.append({
                            "debug": inst.get("debug", 0),
                            "engine": inst["engine"],
                            "ins": [],
                            "name": f"WSPLIT-{ctr}",
                            "opcode": "NoOp",
                            "outs": [],
                            "sync_info": {"on_update": [], "on_wait": [w]},
                        })
                    si["on_wait"] = [waits[-1]]
                out.append(inst)
            bb["instructions"] = out
    nc.m = bass_rust.module_from_json_string(json.dumps(d))


def _rope_tables():
    inv_freq = 1.0 / (ROPE_BASE ** (np.arange(0, Hd, 2, dtype=np.float32) / Hd))
    t = np.arange(S, dtype=np.float32)
    freqs = np.outer(t, inv_freq).astype(np.float32)
    emb = np.concatenate([freqs, freqs], axis=-1)       # (S, Hd)
    cos = np.cos(emb).astype(NPBF16)
    sin = np.sin(emb).astype(NPBF16)
    return np.ascontiguousarray(cos.T), np.ascontiguousarray(sin.T)  # (Hd, S)


def _b64(ap):
    """[128, 4] -> [128, 4, 64] broadcast along a new inner dim."""
    return ap.unsqueeze(2).broadcast_to((128, 4, Hd))


def build_module():
    nc = bass.Bass(trn_type="TRN2")

    x_d = nc.dram_tensor("x", [S, D], BF16, kind="ExternalInput")
    wq_d = nc.dram_tensor("wq", [D, 512], BF16, kind="ExternalInput")
    wk_d = nc.dram_tensor("wk", [D, 512], BF16, kind="ExternalInput")
    wv_d = nc.dram_tensor("wv", [D, THDC], BF16, kind="ExternalInput")
    wo_d = nc.dram_tensor("wo", [THDC, D], BF16, kind="ExternalInput")
    lam_d = nc.dram_tensor("lam", [5, HPC], F32, kind="ExternalInput")
    gnw_d = nc.dram_tensor("gnw", [THDC], F32, kind="ExternalInput")
    gnb_d = nc.dram_tensor("gnb", [THDC], F32, kind="ExternalInput")
    cos_d = nc.dram_tensor("cosT", [Hd, S], BF16, kind="ExternalInput")
    sin_d = nc.dram_tensor("ssinT", [Hd, S], BF16, kind="ExternalInput")
    y_d = nc.dram_tensor("y", [S, D], F32, kind="ExternalOutput")

    ident_d = nc.inline_tensor(np.eye(128, dtype=np.float32).astype(NPBF16),
                               name="identity")
    identf_d = nc.inline_tensor(np.eye(128, dtype=np.float32), name="identityf")
    # rotate_half as a 128x128 +-1 permutation (block-diag per 64-row head):
    # perm @ t gives rh(t); transposed for the lhsT slot
    _m64 = np.zeros((64, 64), dtype=np.float32)
    for _dd in range(32):
        _m64[_dd, _dd + 32] = -1.0
        _m64[_dd + 32, _dd] = 1.0
    _m128 = np.zeros((128, 128), dtype=np.float32)
    _m128[:64, :64] = _m64
    _m128[64:, 64:] = _m64
    rotm_d = nc.inline_tensor(np.ascontiguousarray(_m128.T).astype(NPBF16),
                              name="rotm")

    dma = nc.sync.dma_start      # HWDGE on SP
    dmaT = nc.sync.dma_start_transpose

    with tile.TileContext(nc) as tc:
        with tc.tile_pool(name="persist", bufs=1) as pA, \
             tc.tile_pool(name="dscr", bufs=1, space="DRAM") as pD, \
             tc.tile_pool(name="tsin", bufs=3) as pT, \
             tc.tile_pool(name="e", bufs=4) as pE, \
             tc.tile_pool(name="ow", bufs=4) as pOw, \
             tc.tile_pool(name="gn", bufs=2) as pG, \
             tc.tile_pool(name="ost", bufs=3) as pOst, \
             tc.tile_pool(name="ost0", bufs=8) as pOst0, \
             tc.tile_pool(name="ps_v", bufs=1, space="PSUM") as pVout:

            # ---- persistent SBUF ----
            xT = [pA.tile([128, S], BF16, tag=f"xT{i}", name=f"xT{i}")
                  for i in range(DK)]
            qt = [pA.tile([128, S], BF16, tag=f"qt{i}", name=f"qt{i}") for i in range(4)]
            kt = [pA.tile([128, S], BF16, tag=f"kt{i}", name=f"kt{i}") for i in range(4)]
            vaug = [pA.tile([128, HPC * (Hd + 1)], BF16, tag=f"va{i}", name=f"va{i}")
                    for i in range(NT)]
            yt = [pA.tile([128, S], BF16, tag=f"yt{i}", name=f"yt{i}") for i in range(2)]
            cos2h = pA.tile([128, S], BF16, tag="cos2h", name="cos2h")
            sin2h = pA.tile([128, S], BF16, tag="sin2h", name="sin2h")
            wq_all = pA.tile([128, DK * 512], BF16, tag="wqa", name="wqa")
            wk_all = pA.tile([128, DK * 512], BF16, tag="wka", name="wka")
            wv_all = pA.tile([128, DK * THDC], BF16, tag="wva", name="wva")
            wq_sb = [wq_all[:, i * 512:(i + 1) * 512] for i in range(DK)]
            wk_sb = [wk_all[:, i * 512:(i + 1) * 512] for i in range(DK)]
            wv_sb = [wv_all[:, i * THDC:(i + 1) * THDC] for i in range(DK)]
            wo_sb = [pA.tile([128, D], BF16, tag=f"wo{i}", name=f"wo{i}")
                     for i in range(2)]
            ident = pA.tile([128, 128], BF16, tag="ident", name="ident")
            identf = pA.tile([128, 128], F32, tag="identf", name="identf")
            rotm = pA.tile([128, 128], BF16, tag="rotm", name="rotm")
            ones4 = pA.tile([128, 4], BF16, tag="ones4", name="ones4")

            # ---- DMAs: ordered so V-proj inputs (wv + x^T) land first;
            # one full-S xbar transpose per dk keeps the dispatch count low
            nc.vector.memset(ones4, 1.0)
            dma(out=wv_all[:, :].rearrange("p (dk c) -> p dk c", dk=DK),
                in_=wv_d[:, :].rearrange("(dk p) c -> p dk c", p=128))
            for dk in range(DK):
                dmaT(out=xT[dk][:, 0:1024], in_=x_d[0:1024, dk * 128:(dk + 1) * 128])
            dma(out=wk_all[:, :].rearrange("p (dk c) -> p dk c", dk=DK),
                in_=wk_d[:, :].rearrange("(dk p) c -> p dk c", p=128))
            dma(out=sin2h[0:64, :], in_=sin_d[:, :])
            dma(out=sin2h[64:128, :], in_=sin_d[:, :])
            dma(out=cos2h[0:64, :], in_=cos_d[:, :])
            dma(out=cos2h[64:128, :], in_=cos_d[:, :])
            dma(out=rotm, in_=rotm_d.ap())
            for dk in range(DK):
                dmaT(out=xT[dk][:, 1024:2048],
                     in_=x_d[1024:2048, dk * 128:(dk + 1) * 128])
            dma(out=ident, in_=ident_d.ap())
            dma(out=identf, in_=identf_d.ap())
            dma(out=wq_all[:, :].rearrange("p (dk c) -> p dk c", dk=DK),
                in_=wq_d[:, :].rearrange("(dk p) c -> p dk c", p=128))
            for ci in range(2):
                dma(out=wo_sb[ci], in_=wo_d[ci * 128:(ci + 1) * 128, :])

            # ---- lambdas, gn scale/bias prep ----
            lamsb = pA.tile([1, 20], F32, tag="lamsb", name="lamsb")
            dma(out=lamsb, in_=lam_d[:, :].rearrange("a b -> (a b)").unsqueeze(0))
            lt = pA.tile([1, 8], F32, tag="lt", name="lt")
            nc.vector.tensor_tensor(out=lt[:, 0:4], in0=lamsb[:, 0:4],
                                    in1=lamsb[:, 4:8], op=MULT)
            nc.vector.tensor_tensor(out=lt[:, 4:8], in0=lamsb[:, 8:12],
                                    in1=lamsb[:, 12:16], op=MULT)
            nc.scalar.activation(out=lt, in_=lt, func=AF.Exp)
            lamv = pA.tile([1, 4], F32, tag="lamv", name="lamv")
            nc.vector.tensor_tensor(out=lamv, in0=lt[:, 0:4], in1=lt[:, 4:8], op=SUB)
            nc.vector.tensor_tensor(out=lamv, in0=lamv, in1=lamsb[:, 16:20], op=ADD)
            slam = pA.tile([1, 1], F32, tag="slam", name="slam")
            nc.scalar.activation(out=slam, in_=lamsb[:, 16:17], func=AF.Identity,
                                 scale=-1.0, bias=1.0)
            dscr = pD.tile([1, 8], F32, name="dscr")
            dma(out=dscr[0:1, 0:4], in_=lamv)
            dma(out=dscr[0:1, 4:5], in_=slam)
            lam_b = pA.tile([128, 4], F32, tag="lam_b", name="lam_b")
            dma(out=lam_b, in_=dscr[0:1, 0:4].squeeze(0).partition_broadcast(128))
            slam_b = pA.tile([128, 1], F32, tag="slam_b", name="slam_b")
            dma(out=slam_b, in_=dscr[0:1, 4:5].squeeze(0).partition_broadcast(128))
            gnw_eff = pA.tile([128, THDC], F32, tag="gnw_eff", name="gnw_eff")
            dma(out=gnw_eff, in_=gnw_d.ap().partition_broadcast(128))
            nc.vector.tensor_scalar_mul(gnw_eff, gnw_eff, slam_b[:, 0:1])
            gnb_eff = pA.tile([128, THDC], F32, tag="gnb_eff", name="gnb_eff")
            dma(out=gnb_eff, in_=gnb_d.ap().partition_broadcast(128))
            nc.vector.tensor_scalar_mul(gnb_eff, gnb_eff, slam_b[:, 0:1])
            lam_bh = []
            for h in range(HPC):
                lb = pA.tile([128, 4], F32, tag=f"lam_bh{h}", name=f"lam_bh{h}")
                nc.vector.tensor_copy(out=lb,
                                      in_=lam_b[:, h:h + 1].broadcast_to((128, 4)))
                lam_bh.append(lb)

            # ---- task generators ----
            def g_proj(w_sb, dst, tt, c, idx, qpool, qtag, ppool, ptag):
                cs = slice(c * 512, (c + 1) * 512)
                psq = qpool.tile([128, 512], F32, tag=qtag, name=f"psq{idx}")
                for i in range(DK):
                    dk = (i + tt * 2 + c) % DK
                    nc.tensor.matmul(psq, lhsT=w_sb[dk][:, tt * 128:(tt + 1) * 128],
                                     rhs=xT[dk][:, cs],
                                     start=(i == 0), stop=(i == DK - 1))
                    yield
                tsin = pT.tile([128, 512], BF16, tag="tsin", name=f"ts{idx}")
                nc.vector.tensor_tensor(out=tsin, in0=psq, in1=sin2h[:, cs], op=MULT)
                yield
                yield
                perm = ppool.tile([128, 512], F32, tag=ptag, name=f"pm{idx}")
                nc.tensor.matmul(perm, lhsT=rotm, rhs=tsin, start=True, stop=True)
                yield
                nc.vector.tensor_tensor(out=dst[tt][:, cs], in0=psq,
                                        in1=cos2h[:, cs], op=MULT)
                nc.vector.tensor_tensor(out=dst[tt][:, cs], in0=dst[tt][:, cs],
                                        in1=perm, op=ADD)

            def g_vproj(st):
                # single full-region tile per group: the whole-bank pending-zero
                # of start=True makes sub-bank double-buffering unsafe, so rely
                # on the WAR dep against the previous group's copy instead
                psv = pVout.tile([128, 512], F32, tag="vout", name=f"psv{st}")[:, 0:THDC]
                for i in range(DK):
                    dk = (i + st) % DK
                    nc.tensor.matmul(psv, lhsT=xT[dk][:, st * 128:(st + 1) * 128],
                                     rhs=wv_sb[dk], start=(i == 0), stop=(i == DK - 1))
                    yield
                va = vaug[st]
                vav = va[:, :].rearrange("p (h c) -> p h c", h=HPC)
                nc.vector.tensor_copy(
                    out=vav[:, :, 0:Hd],
                    in_=psv.rearrange("p (h c) -> p h c", h=HPC))
                rv = vav[:, :, Hd:Hd + 1].rearrange("p h c -> p (h c)")
                nc.vector.tensor_copy(out=rv, in_=ones4)

            def g_py(c, h, dtb, ppool):
                # delay so the GN chain (which produces dtb) completes before
                # the first transpose reaches the head of the PE queue
                for _ in range(8):
                    yield
                ro = (h % 2) * 64
                for stl in range(4):
                    st = c * 4 + stl
                    ptr = ppool.tile([128, 512], F32, tag="perm",
                                     name=f"py{c}{h}{stl}")[0:64, 0:128]
                    nc.tensor.transpose(ptr, dtb[:, stl * Hd:(stl + 1) * Hd], identf)
                    yield
                    nc.vector.tensor_copy(
                        out=yt[h // 2][ro:ro + 64, st * 128:(st + 1) * 128], in_=ptr)

            ost0_tiles = {}

            def g_oproj_ci0(st, oc):
                pout = pVout.tile([128, 512], F32, tag="vout", name=f"pc0{st}_{oc}")
                nc.tensor.matmul(pout, lhsT=yt[0][:, st * 128:(st + 1) * 128],
                                 rhs=wo_sb[0][:, oc * 512:(oc + 1) * 512],
                                 start=True, stop=True)
                yield
                o0 = pOst0.tile([128, 512], F32, tag="ost0", name=f"o0{st}_{oc}")
                nc.vector.tensor_copy(out=o0, in_=pout)
                ost0_tiles[(st, oc)] = o0

            def g_oproj_ci1(st, oc, pool, tag):
                pout = pool.tile([128, 512], F32, tag=tag, name=f"pc1{st}_{oc}")
                nc.tensor.matmul(pout, lhsT=yt[1][:, st * 128:(st + 1) * 128],
                                 rhs=wo_sb[1][:, oc * 512:(oc + 1) * 512],
                                 start=True, stop=True)
                yield
                ost = pOst.tile([128, 512], F32, tag="ost", name=f"ost{st}_{oc}")
                nc.vector.tensor_tensor(out=ost, in0=pout,
                                        in1=ost0_tiles[(st, oc)], op=ADD)
                dma(out=y_d[st * 128:(st + 1) * 128, oc * 512:(oc + 1) * 512],
                    in_=ost)

            def g_oproj(c, st, oc, pool, tag):
                pout = pool.tile([128, 512], F32, tag=tag, name=f"po{st}_{oc}")
                for ci in range(2):
                    nc.tensor.matmul(pout, lhsT=yt[ci][:, st * 128:(st + 1) * 128],
                                     rhs=wo_sb[ci][:, oc * 512:(oc + 1) * 512],
                                     start=(ci == 0), stop=(ci == 1))
                    yield
                ost = pOst.tile([128, 512], F32, tag="ost", name=f"ost{st}_{oc}")
                nc.vector.tensor_copy(out=ost, in_=pout)
                dma(out=y_d[st * 128:(st + 1) * 128, oc * 512:(oc + 1) * 512],
                    in_=ost)

            # two-priority task queues: prim (py/o_proj, pulled first),
            # sec (K/Q proj with ordering markers)
            prim = deque()
            sec = deque()

            def drain_until(key):
                # run everything in prim+sec up to (and incl.) marker `key`
                while True:
                    q = prim if prim else sec
                    if not q:
                        return
                    obj = q[0]
                    if isinstance(obj, str):
                        sec.popleft()
                        if obj == key:
                            return
                        continue
                    try:
                        next(obj)
                    except StopIteration:
                        q.popleft()

            def drain_all():
                while prim or sec:
                    q = prim if prim else sec
                    obj = q[0]
                    if isinstance(obj, str):
                        q.popleft()
                        continue
                    try:
                        next(obj)
                    except StopIteration:
                        q.popleft()

            rr = [0]

            def pull2(n=1, use_prim=True):
                emitted = 0
                while emitted < n:
                    rr[0] ^= 1
                    qs = [prim, sec] if (rr[0] and use_prim) else [sec, prim]
                    if not use_prim:
                        qs = [sec]
                    q = None
                    for cand in qs:
                        if cand is sec and (not sec or isinstance(sec[0], str)):
                            continue
                        if cand:
                            q = cand
                            break
                    if q is None:
                        return
                    try:
                        next(q[0])
                        emitted += 1
                    except StopIteration:
                        q.popleft()

            # ---- prelude: V proj + K(tt0/tt2) + Q(c0, tt0/tt2), interleaved
            # at group granularity so rope DVE reads overlap the next group ----
            with tc.tile_pool(name="ps_pre", bufs=4, space="PSUM") as pPre:
                def merge(pre, kq_pre):
                    # merge V and K/Q streams ~3 PE instructions at a time so
                    # rope DVE latency hides behind V-proj matmuls
                    while pre or kq_pre:
                        for q in (pre, kq_pre):
                            if not q:
                                continue
                            g = q[0]
                            done = False
                            for _ in range(3):
                                try:
                                    next(g)
                                except StopIteration:
                                    done = True
                                    break
                            if done:
                                q.popleft()

                # phase A: everything that only needs x columns 0..1023
                preA = deque(g_vproj(st) for st in range(8))
                kqA = deque()
                for c in (0, 1):
                    kqA.append(g_proj(wk_sb, kt, 0, c, f"k0{c}", pPre, "pp", pPre, "pp"))
                for c in (0, 1):
                    kqA.append(g_proj(wk_sb, kt, 2, c, f"k2{c}", pPre, "pp", pPre, "pp"))
                kqA.append(g_proj(wq_sb, qt, 0, 0, "q00", pPre, "pp", pPre, "pp"))
                kqA.append(g_proj(wq_sb, qt, 2, 0, "q20", pPre, "pp", pPre, "pp"))
                merge(preA, kqA)
                # phase B: second x half
                preB = deque(g_vproj(st) for st in range(8, NT))
                kqB = deque()
                for c in (2, 3):
                    kqB.append(g_proj(wk_sb, kt, 0, c, f"k0{c}", pPre, "pp", pPre, "pp"))
                for c in (2, 3):
                    kqB.append(g_proj(wk_sb, kt, 2, c, f"k2{c}", pPre, "pp", pPre, "pp"))
                merge(preB, kqB)

            # ---- attention + group norm (prelude PSUM pool closed) ----
            with tc.tile_pool(name="ps_pv", bufs=2, space="PSUM") as pPv, \
                 tc.tile_pool(name="ps_q", bufs=1, space="PSUM") as pPq, \
                 tc.tile_pool(name="ps_perm", bufs=1, space="PSUM") as pPerm, \
                 tc.tile_pool(name="ps_s", bufs=3, space="PSUM") as pPs:
                for c in range(NCH):
                    sec.append(g_proj(wk_sb, kt, 1, c, f"k1{c}", pPq, "psq", pPerm, "perm"))
                for c in range(NCH):
                    sec.append(g_proj(wk_sb, kt, 3, c, f"k3{c}", pPq, "psq", pPerm, "perm"))
                sec.append(g_proj(wq_sb, qt, 1, 0, "q10", pPq, "psq", pPerm, "perm"))
                sec.append(g_proj(wq_sb, qt, 3, 0, "q30", pPq, "psq", pPerm, "perm"))
                sec.append("c0h2")
                for c in range(1, NCH):
                    sec.append(g_proj(wq_sb, qt, 0, c, f"q0{c}", pPq, "psq", pPerm, "perm"))
                    sec.append(g_proj(wq_sb, qt, 2, c, f"q2{c}", pPq, "psq", pPerm, "perm"))
                    sec.append(f"c{c}h0")
                    sec.append(g_proj(wq_sb, qt, 1, c, f"q1{c}", pPq, "psq", pPerm, "perm"))
                    sec.append(g_proj(wq_sb, qt, 3, c, f"q3{c}", pPq, "psq", pPerm, "perm"))
                    sec.append(f"c{c}h2")
                for c in range(NCH):
                    cs = slice(c * 512, (c + 1) * 512)
                    for h in range(HPC):
                        if h == 0 and c > 0:
                            drain_until(f"c{c}h0")
                        if h == 2:
                            drain_until(f"c{c}h2")
                        ro = (h % 2) * 64
                        ows = []
                        for br, tt in ((0, h // 2), (1, 2 + h // 2)):
                            pv = pPv.tile([128, 4, 65], F32, tag="pv",
                                          name=f"pv{c}{h}{br}")
                            prev_e = None
                            for sk in range(NT):
                                pss = pPs.tile([128, 512], F32, tag="pss",
                                               name=f"pss{c}{h}{br}{sk}")
                                nc.tensor.matmul(
                                    pss,
                                    lhsT=kt[tt][ro:ro + 64, sk * 128:(sk + 1) * 128],
                                    rhs=qt[tt][ro:ro + 64, cs],
                                    start=True, stop=True)
                                e = pE.tile([128, 512], BF16, tag="e",
                                            name=f"e{c}{h}{br}{sk}")
                                nc.scalar.activation(out=e, in_=pss, func=AF.Exp,
                                                     scale=QK_SCALE)
                                if prev_e is not None:
                                    va_h = vaug[sk - 1][:, h * 65:(h + 1) * 65]
                                    for qs in range(4):
                                        # start=True marks the whole PSUM bank
                                        # pending-zero, so only the first
                                        # matmul of the bank may set it
                                        nc.tensor.matmul(
                                            pv[:, qs, :],
                                            lhsT=prev_e[:, qs * 128:(qs + 1) * 128],
                                            rhs=va_h,
                                            start=(sk - 1 == 0 and qs == 0),
                                            stop=False)
                                prev_e = e
                                pull2(2, use_prim=(br == 1))
                            va_h = vaug[NT - 1][:, h * 65:(h + 1) * 65]
                            for qs in range(4):
                                nc.tensor.matmul(
                                    pv[:, qs, :],
                                    lhsT=prev_e[:, qs * 128:(qs + 1) * 128],
                                    rhs=va_h, start=False, stop=True)
                            ow = pOw.tile([128, 4, 65], F32, tag="ow",
                                          name=f"ow{c}{h}{br}")
                            nc.vector.tensor_copy(out=ow, in_=pv)
                            ows.append(ow)

                        # group norm for this head (4 s-tiles = the 4 "groups")
                        ow1, ow2 = ows
                        r1 = ow1[:, :, Hd]
                        r2 = ow2[:, :, Hd]
                        rec = pG.tile([128, 4], F32, tag="rec", name=f"rec{c}{h}")
                        nc.vector.reciprocal(rec, r2)
                        rho = pG.tile([128, 4], F32, tag="rho", name=f"rho{c}{h}")
                        nc.vector.tensor_tensor(out=rho, in0=r1, in1=rec, op=MULT)
                        nc.vector.tensor_tensor(out=rho, in0=rho, in1=lam_bh[h], op=MULT)
                        dt_ = pG.tile([128, THDC], F32, tag="dt", name=f"dt{c}{h}")
                        dtv = dt_[:, :].rearrange("p (g d) -> p g d", g=4)
                        nc.vector.tensor_tensor(out=dtv, in0=ow2[:, :, 0:Hd],
                                                in1=_b64(rho), op=MULT)
                        nc.vector.tensor_tensor(out=dtv, in0=ow1[:, :, 0:Hd],
                                                in1=dtv, op=SUB)
                        s1 = pG.tile([128, 4], F32, tag="s1", name=f"s1{c}{h}")
                        nc.vector.reduce_sum(out=s1, in_=dtv, axis=AX)
                        nc.vector.tensor_scalar_mul(s1, s1, -1.0 / Hd)
                        nc.vector.tensor_tensor(out=dtv, in0=dtv, in1=_b64(s1), op=ADD)
                        d2 = pG.tile([128, THDC], F32, tag="d2", name=f"d2{c}{h}")
                        nc.vector.tensor_tensor(out=d2, in0=dt_, in1=dt_, op=MULT)
                        s2 = pG.tile([128, 4], F32, tag="s2", name=f"s2{c}{h}")
                        nc.vector.reduce_sum(
                            out=s2, in_=d2[:, :].rearrange("p (g d) -> p g d", g=4),
                            axis=AX)
                        nc.vector.tensor_tensor(out=rec, in0=r1, in1=r1, op=MULT)
                        nc.vector.tensor_scalar_mul(rec, rec, GN_EPS)
                        nc.vector.tensor_scalar_mul(s2, s2, 1.0 / Hd)
                        nc.vector.tensor_tensor(out=s2, in0=s2, in1=rec, op=ADD)
                        nc.scalar.activation(out=s2, in_=s2, func=AF.Sqrt)
                        nc.vector.reciprocal(s2, s2)
                        nc.vector.tensor_tensor(out=dtv, in0=dtv, in1=_b64(s2), op=MULT)
                        gw = gnw_eff[:, h * Hd:(h + 1) * Hd].unsqueeze(1) \
                            .broadcast_to((128, 4, Hd))
                        gb = gnb_eff[:, h * Hd:(h + 1) * Hd].unsqueeze(1) \
                            .broadcast_to((128, 4, Hd))
                        nc.vector.tensor_tensor(out=dtv, in0=dtv, in1=gw, op=MULT)
                        nc.vector.tensor_tensor(out=dtv, in0=dtv, in1=gb, op=ADD)
                        prim.append(g_py(c, h, dt_, pPerm))
                        if h == HPC - 1 and c < NCH - 1:
                            for stl in range(4):
                                st = c * 4 + stl
                                for oc in range(2):
                                    prim.append(g_oproj(c, st, oc, pVout, "vout"))
                        if h == 1 and c == NCH - 1:
                            for stl in range(4):
                                st = c * 4 + stl
                                for oc in range(2):
                                    prim.append(g_oproj_ci0(st, oc))
                drain_all()
            # pPs closed: 2 banks free for double-buffered tail o_proj
            with tc.tile_pool(name="ps_tail", bufs=2, space="PSUM") as pTail:
                tail = []
                for stl in range(4):
                    st = (NCH - 1) * 4 + stl
                    for oc in range(2):
                        tail.append(g_oproj_ci1(st, oc, pTail, "tail"))
                # interleave pairs of groups so the two PSUM slots overlap
                for i in range(0, len(tail), 2):
                    alive = list(tail[i:i + 2])
                    while alive:
                        for g in list(alive):
                            try:
                                next(g)
                            except StopIteration:
                                alive.remove(g)

    _split_multi_waits(nc)
    return nc


_CACHE = {}


def _get_module():
    if "nc" not in _CACHE:
        _CACHE["nc"] = build_module()
        _CACHE["tables"] = _rope_tables()
    return _CACHE["nc"], _CACHE["tables"]


def kernel(x, Wq, Wk, Wv, Wo, lambda_q1, lambda_k1, lambda_q2, lambda_k2,
           lambda_init, gn_weight, gn_bias):
    from concourse.bass_utils import run_bass_kernel_spmd

    x = np.asarray(x, dtype=np.float32)
    Wq = np.asarray(Wq, dtype=np.float32)
    Wk = np.asarray(Wk, dtype=np.float32)
    Wv = np.asarray(Wv, dtype=np.float32)
    Wo = np.asarray(Wo, dtype=np.float32)
    lq1 = np.asarray(lambda_q1, dtype=np.float32)
    lk1 = np.asarray(lambda_k1, dtype=np.float32)
    lq2 = np.asarray(lambda_q2, dtype=np.float32)
    lk2 = np.asarray(lambda_k2, dtype=np.float32)
    lam_init = np.float32(np.asarray(lambda_init).reshape(()))
    gnw = np.asarray(gn_weight, dtype=np.float32)
    gnb = np.asarray(gn_bias, dtype=np.float32)

    nc, (cosT, ssinT) = _get_module()

    xb = [np.ascontiguousarray(x[b].astype(NPBF16)) for b in range(B)]
    in_maps = []
    for core in range(8):
        b = core // 4
        hb = (core % 4) * HPC
        c1 = slice(hb * Hd, (hb + HPC) * Hd)
        c2 = slice(H * Hd + hb * Hd, H * Hd + (hb + HPC) * Hd)
        lam = np.stack([lq1[hb:hb + HPC], lk1[hb:hb + HPC],
                        lq2[hb:hb + HPC], lk2[hb:hb + HPC],
                        np.full(HPC, lam_init, np.float32)]).astype(np.float32)
        in_maps.append({
            "x": xb[b],
            "wq": np.ascontiguousarray(
                np.concatenate([Wq[:, c1], Wq[:, c2]], axis=1).astype(NPBF16)),
            "wk": np.ascontiguousarray(
                np.concatenate([Wk[:, c1], Wk[:, c2]], axis=1).astype(NPBF16)),
            "wv": np.ascontiguousarray(Wv[:, c1].astype(NPBF16)),
            "wo": np.ascontiguousarray(Wo[c1, :].astype(NPBF16)),
            "lam": lam,
            "gnw": np.ascontiguousarray(gnw[c1]),
            "gnb": np.ascontiguousarray(gnb[c1]),
            "cosT": cosT,
            "ssinT": ssinT,
        })

    last_err = None
    for attempt in range(3):
        try:
            res = run_bass_kernel_spmd(nc, in_maps, core_ids=list(range(8)))
            break
        except Exception as e:  # transient axon/device hiccups
            last_err = e
            time.sleep(10 * (attempt + 1))
    else:
        raise last_err

    out = np.zeros((B, S, D), dtype=np.float32)
    for core in range(8):
        out[core // 4] += res.results[core]["y"]
    return out


# revision 32
# speedup vs baseline: 1.2591x; 1.0004x over previous
"""Differential attention (nn_DifferentialAttention_84679575208071) on 8 TRN2
NeuronCores via Bass/Tile.

Sharding: hybrid data-parallel x tensor-parallel. Core c handles batch c//4 and
heads 4*(c%4) .. 4*(c%4)+4 (B=2, H=16 -> 2 batch groups x 4-way head split).
Each core computes its 4 heads' attention + group-norm + partial o_proj; the
host sums the 4 partial (S, D) outputs per batch and stacks the 2 batches.

Per-core pipeline (all matmuls bf16 = 1 col/cycle on the PE):
  xT        via DMA-transpose (xbar) straight from DRAM -> SBUF, no PE work
  qT/kT     = (W slice)^T x^T, rope applied with the rotate_half PE perm fed
              from a (q*sin) product so no PSUM->SBUF stage copy is needed
  v         = x Wv (row layout) + a ones column per head so the PV matmul
              also yields softmax denominators
  attention chunk-major, per (chunk, head, branch): 16 score matmuls
              [128kpos x 512q] -> exp on Act (bf16 out) -> PV in the FLIPPED
              orientation out[q, ch]: lhsT = e-subtile, rhs = v-aug; this
              halves PV streaming cost and lands the output already in
              [s-partition, channel] layout for group-norm (no transposes)
  diff      = O1 - (lam*r1/r2) * O2 folded through group-norm (eps scaled by
              r1^2), then gn_weight*(1-lam_init) + bias, o_proj per chunk
  scheduling: projection / o_proj / output-transpose work is interleaved into
              the attention stream via a task queue pulled once per score tile
              so the PE never idles while the Act engine streams exps.
"""
import json
import time
from collections import deque

import numpy as np
import ml_dtypes

import bass_rust
import concourse.bass as bass
import concourse.mybir as mybir
import concourse.tile as tile

F32 = mybir.dt.float32
BF16 = mybir.dt.bfloat16
MULT = mybir.AluOpType.mult
ADD = mybir.AluOpType.add
SUB = mybir.AluOpType.subtract
AX = mybir.AxisListType.X
AF = mybir.ActivationFunctionType

B, S, D = 2, 2048, 1024
H, Hd = 16, 64
HPC = 4               # heads per core
THDC = HPC * Hd       # 256 channels per core
NT = S // 128         # 16 s-tiles
NCH = S // 512        # 4 chunks
DK = D // 128         # 8 contraction tiles
LAMBDA_INIT = 0.8
GN_EPS = 1e-5
ROPE_BASE = 10000.0
QK_SCALE = float(Hd) ** -0.5  # 0.125

NPBF16 = ml_dtypes.bfloat16


def _split_multi_waits(nc):
    """This container's walrus rejects >1 sync wait per instruction. Hoist
    extra waits onto same-engine NoOps inserted right before the instruction
    (engine queues are in-order, so semantics are unchanged)."""
    d = json.loads(bass_rust.module_to_json_string(nc.m))
    ctr = 0
    for f in d["functions"]:
        for bb in f["blocks"]:
            out = []
            for inst in bb.get("instructions", []):
                si = inst.get("sync_info")
                waits = si.get("on_wait", []) if si else []
                if len(waits) > 1:
                    for w in waits[:-1]:
                        ctr += 1
                        out.append({
                            "debug": inst.get("debug", 0),
                            "engine": inst["engine"],
                            "ins": [],
                            "name": f"WSPLIT-{ctr}",
                            "opcode": "NoOp",
                            "outs": [],
                            "sync_info": {"on_update": [], "on_wait": [w]},
                        })
                    si["on_wait"] = [waits[-1]]
                out.append(inst)
            bb["instructions"] = out
    nc.m = bass_rust.module_from_json_string(json.dumps(d))


def _rope_tables():
    inv_freq = 1.0 / (ROPE_BASE ** (np.arange(0, Hd, 2, dtype=np.float32) / Hd))
    t = np.arange(S, dtype=np.float32)
    freqs = np.outer(t, inv_freq).astype(np.float32)
    emb = np.concatenate([freqs, freqs], axis=-1)       # (S, Hd)
    cos = np.cos(emb).astype(NPBF16)
    sin = np.sin(emb).astype(NPBF16)
    return np.ascontiguousarray(cos.T), np.ascontiguousarray(sin.T)  # (Hd, S)


def _b64(ap):
    """[128, 4] -> [128, 4, 64] broadcast along a new inner dim."""
    return ap.unsqueeze(2).broadcast_to((128, 4, Hd))


def build_module():
    nc = bass.Bass(trn_type="TRN2")

    x_d = nc.dram_tensor("x", [S, D], BF16, kind="ExternalInput")
    wq_d = nc.dram_tensor("wq", [D, 512], BF16, kind="ExternalInput")
    wk_d = nc.dram_tensor("wk", [D, 512], BF16, kind="ExternalInput")
    wv_d = nc.dram_tensor("wv", [D, THDC], BF16, kind="ExternalInput")
    wo_d = nc.dram_tensor("wo", [THDC, D], BF16, kind="ExternalInput")
    lam_d = nc.dram_tensor("lam", [5, HPC], F32, kind="ExternalInput")
    gnw_d = nc.dram_tensor("gnw", [THDC], F32, kind="ExternalInput")
    gnb_d = nc.dram_tensor("gnb", [THDC], F32, kind="ExternalInput")
    cos_d = nc.dram_tensor("cosT", [Hd, S], BF16, kind="ExternalInput")
    sin_d = nc.dram_tensor("ssinT", [Hd, S], BF16, kind="ExternalInput")
    y_d = nc.dram_tensor("y", [S, D], F32, kind="ExternalOutput")

    ident_d = nc.inline_tensor(np.eye(128, dtype=np.float32).astype(NPBF16),
                               name="identity")
    identf_d = nc.inline_tensor(np.eye(128, dtype=np.float32), name="identityf")
    # rotate_half as a 128x128 +-1 permutation (block-diag per 64-row head):
    # perm @ t gives rh(t); transposed for the lhsT slot
    _m64 = np.zeros((64, 64), dtype=np.float32)
    for _dd in range(32):
        _m64[_dd, _dd + 32] = -1.0
        _m64[_dd + 32, _dd] = 1.0
    _m128 = np.zeros((128, 128), dtype=np.float32)
    _m128[:64, :64] = _m64
    _m128[64:, 64:] = _m64
    rotm_d = nc.inline_tensor(np.ascontiguousarray(_m128.T).astype(NPBF16),
                              name="rotm")

    dma = nc.sync.dma_start      # HWDGE on SP
    dmaT = nc.sync.dma_start_transpose

    with tile.TileContext(nc) as tc:
        with tc.tile_pool(name="persist", bufs=1) as pA, \
             tc.tile_pool(name="dscr", bufs=1, space="DRAM") as pD, \
             tc.tile_pool(name="tsin", bufs=3) as pT, \
             tc.tile_pool(name="e", bufs=4) as pE, \
             tc.tile_pool(name="ow", bufs=4) as pOw, \
             tc.tile_pool(name="gn", bufs=2) as pG, \
             tc.tile_pool(name="ost", bufs=3) as pOst, \
             tc.tile_pool(name="ost0", bufs=8) as pOst0, \
             tc.tile_pool(name="ps_v", bufs=1, space="PSUM") as pVout:

            # ---- persistent SBUF ----
            xT = [pA.tile([128, S], BF16, tag=f"xT{i}", name=f"xT{i}")
                  for i in range(DK)]
            qt = [pA.tile([128, S], BF16, tag=f"qt{i}", name=f"qt{i}") for i in range(4)]
            kt = [pA.tile([128, S], BF16, tag=f"kt{i}", name=f"kt{i}") for i in range(4)]
            vaug = [pA.tile([128, HPC * (Hd + 1)], BF16, tag=f"va{i}", name=f"va{i}")
                    for i in range(NT)]
            yt = [pA.tile([128, S], BF16, tag=f"yt{i}", name=f"yt{i}") for i in range(2)]
            cos2h = pA.tile([128, S], BF16, tag="cos2h", name="cos2h")
            sin2h = pA.tile([128, S], BF16, tag="sin2h", name="sin2h")
            wq_all = pA.tile([128, DK * 512], BF16, tag="wqa", name="wqa")
            wk_all = pA.tile([128, DK * 512], BF16, tag="wka", name="wka")
            wv_all = pA.tile([128, DK * THDC], BF16, tag="wva", name="wva")
            wq_sb = [wq_all[:, i * 512:(i + 1) * 512] for i in range(DK)]
            wk_sb = [wk_all[:, i * 512:(i + 1) * 512] for i in range(DK)]
            wv_sb = [wv_all[:, i * THDC:(i + 1) * THDC] for i in range(DK)]
            wo_sb = [pA.tile([128, D], BF16, tag=f"wo{i}", name=f"wo{i}")
                     for i in range(2)]
            ident = pA.tile([128, 128], BF16, tag="ident", name="ident")
            identf = pA.tile([128, 128], F32, tag="identf", name="identf")
            rotm = pA.tile([128, 128], BF16, tag="rotm", name="rotm")
            ones4 = pA.tile([128, 4], BF16, tag="ones4", name="ones4")

            # ---- DMAs: ordered so V-proj inputs (wv + x^T) land first;
            # one full-S xbar transpose per dk keeps the dispatch count low
            nc.vector.memset(ones4, 1.0)
            dma(out=wv_all[:, :].rearrange("p (dk c) -> p dk c", dk=DK),
                in_=wv_d[:, :].rearrange("(dk p) c -> p dk c", p=128))
            for dk in range(DK):
                dmaT(out=xT[dk][:, 0:1024], in_=x_d[0:1024, dk * 128:(dk + 1) * 128])
            dma(out=wk_all[:, :].rearrange("p (dk c) -> p dk c", dk=DK),
                in_=wk_d[:, :].rearrange("(dk p) c -> p dk c", p=128))
            dma(out=sin2h[0:64, :], in_=sin_d[:, :])
            dma(out=sin2h[64:128, :], in_=sin_d[:, :])
            dma(out=cos2h[0:64, :], in_=cos_d[:, :])
            dma(out=cos2h[64:128, :], in_=cos_d[:, :])
            dma(out=rotm, in_=rotm_d.ap())
            for dk in range(DK):
                dmaT(out=xT[dk][:, 1024:2048],
                     in_=x_d[1024:2048, dk * 128:(dk + 1) * 128])
            dma(out=ident, in_=ident_d.ap())
            dma(out=identf, in_=identf_d.ap())
            dma(out=wq_all[:, :].rearrange("p (dk c) -> p dk c", dk=DK),
                in_=wq_d[:, :].rearrange("(dk p) c -> p dk c", p=128))
            for ci in range(2):
                dma(out=wo_sb[ci], in_=wo_d[ci * 128:(ci + 1) * 128, :])

            # ---- lambdas, gn scale/bias prep ----
            lamsb = pA.tile([1, 20], F32, tag="lamsb", name="lamsb")
            dma(out=lamsb, in_=lam_d[:, :].rearrange("a b -> (a b)").unsqueeze(0))
            lt = pA.tile([1, 8], F32, tag="lt", name="lt")
            nc.vector.tensor_tensor(out=lt[:, 0:4], in0=lamsb[:, 0:4],
                                    in1=lamsb[:, 4:8], op=MULT)
            nc.vector.tensor_tensor(out=lt[:, 4:8], in0=lamsb[:, 8:12],
                                    in1=lamsb[:, 12:16], op=MULT)
            nc.scalar.activation(out=lt, in_=lt, func=AF.Exp)
            lamv = pA.tile([1, 4], F32, tag="lamv", name="lamv")
            nc.vector.tensor_tensor(out=lamv, in0=lt[:, 0:4], in1=lt[:, 4:8], op=SUB)
            nc.vector.tensor_tensor(out=lamv, in0=lamv, in1=lamsb[:, 16:20], op=ADD)
            slam = pA.tile([1, 1], F32, tag="slam", name="slam")
            nc.scalar.activation(out=slam, in_=lamsb[:, 16:17], func=AF.Identity,
                                 scale=-1.0, bias=1.0)
            dscr = pD.tile([1, 8], F32, name="dscr")
            dma(out=dscr[0:1, 0:4], in_=lamv)
            dma(out=dscr[0:1, 4:5], in_=slam)
            lam_b = pA.tile([128, 4], F32, tag="lam_b", name="lam_b")
            dma(out=lam_b, in_=dscr[0:1, 0:4].squeeze(0).partition_broadcast(128))
            slam_b = pA.tile([128, 1], F32, tag="slam_b", name="slam_b")
            dma(out=slam_b, in_=dscr[0:1, 4:5].squeeze(0).partition_broadcast(128))
            gnw_eff = pA.tile([128, THDC], F32, tag="gnw_eff", name="gnw_eff")
            dma(out=gnw_eff, in_=gnw_d.ap().partition_broadcast(128))
            nc.vector.tensor_scalar_mul(gnw_eff, gnw_eff, slam_b[:, 0:1])
            gnb_eff = pA.tile([128, THDC], F32, tag="gnb_eff", name="gnb_eff")
            dma(out=gnb_eff, in_=gnb_d.ap().partition_broadcast(128))
            nc.vector.tensor_scalar_mul(gnb_eff, gnb_eff, slam_b[:, 0:1])
            lam_bh = []
            for h in range(HPC):
                lb = pA.tile([128, 4], F32, tag=f"lam_bh{h}", name=f"lam_bh{h}")
                nc.vector.tensor_copy(out=lb,
                                      in_=lam_b[:, h:h + 1].broadcast_to((128, 4)))
                lam_bh.append(lb)

            # ---- task generators ----
            def g_proj(w_sb, dst, tt, c, idx, qpool, qtag, ppool, ptag):
                cs = slice(c * 512, (c + 1) * 512)
                psq = qpool.tile([128, 512], F32, tag=qtag, name=f"psq{idx}")
                for i in range(DK):
                    dk = (i + tt * 2 + c) % DK
                    nc.tensor.matmul(psq, lhsT=w_sb[dk][:, tt * 128:(tt + 1) * 128],
                                     rhs=xT[dk][:, cs],
                                     start=(i == 0), stop=(i == DK - 1))
                    yield
                tsin = pT.tile([128, 512], BF16, tag="tsin", name=f"ts{idx}")
                nc.vector.tensor_tensor(out=tsin, in0=psq, in1=sin2h[:, cs], op=MULT)
                yield
                yield
                perm = ppool.tile([128, 512], F32, tag=ptag, name=f"pm{idx}")
                nc.tensor.matmul(perm, lhsT=rotm, rhs=tsin, start=True, stop=True)
                yield
                nc.vector.tensor_tensor(out=dst[tt][:, cs], in0=psq,
                                        in1=cos2h[:, cs], op=MULT)
                nc.vector.tensor_tensor(out=dst[tt][:, cs], in0=dst[tt][:, cs],
                                        in1=perm, op=ADD)

            def g_vproj(st):
                # single full-region tile per group: the whole-bank pending-zero
                # of start=True makes sub-bank double-buffering unsafe, so rely
                # on the WAR dep against the previous group's copy instead
                psv = pVout.tile([128, 512], F32, tag="vout", name=f"psv{st}")[:, 0:THDC]
                for i in range(DK):
                    dk = (i + st) % DK
                    nc.tensor.matmul(psv, lhsT=xT[dk][:, st * 128:(st + 1) * 128],
                                     rhs=wv_sb[dk], start=(i == 0), stop=(i == DK - 1))
                    yield
                va = vaug[st]
                vav = va[:, :].rearrange("p (h c) -> p h c", h=HPC)
                nc.vector.tensor_copy(
                    out=vav[:, :, 0:Hd],
                    in_=psv.rearrange("p (h c) -> p h c", h=HPC))
                rv = vav[:, :, Hd:Hd + 1].rearrange("p h c -> p (h c)")
                nc.vector.tensor_copy(out=rv, in_=ones4)

            def g_py(c, h, dtb, ppool):
                # delay so the GN chain (which produces dtb) completes before
                # the first transpose reaches the head of the PE queue
                for _ in range(8):
                    yield
                ro = (h % 2) * 64
                for stl in range(4):
                    st = c * 4 + stl
                    ptr = ppool.tile([128, 512], F32, tag="perm",
                                     name=f"py{c}{h}{stl}")[0:64, 0:128]
                    nc.tensor.transpose(ptr, dtb[:, stl * Hd:(stl + 1) * Hd], identf)
                    yield
                    nc.vector.tensor_copy(
                        out=yt[h // 2][ro:ro + 64, st * 128:(st + 1) * 128], in_=ptr)

            ost0_tiles = {}

            def g_oproj_ci0(st, oc):
                pout = pVout.tile([128, 512], F32, tag="vout", name=f"pc0{st}_{oc}")
                nc.tensor.matmul(pout, lhsT=yt[0][:, st * 128:(st + 1) * 128],
                                 rhs=wo_sb[0][:, oc * 512:(oc + 1) * 512],
                                 start=True, stop=True)
                yield
                o0 = pOst0.tile([128, 512], F32, tag="ost0", name=f"o0{st}_{oc}")
                nc.vector.tensor_copy(out=o0, in_=pout)
                ost0_tiles[(st, oc)] = o0

            def g_oproj_ci1(st, oc, pool, tag):
                pout = pool.tile([128, 512], F32, tag=tag, name=f"pc1{st}_{oc}")
                nc.tensor.matmul(pout, lhsT=yt[1][:, st * 128:(st + 1) * 128],
                                 rhs=wo_sb[1][:, oc * 512:(oc + 1) * 512],
                                 start=True, stop=True)
                yield
                ost = pOst.tile([128, 512], F32, tag="ost", name=f"ost{st}_{oc}")
                nc.vector.tensor_tensor(out=ost, in0=pout,
                                        in1=ost0_tiles[(st, oc)], op=ADD)
                dma(out=y_d[st * 128:(st + 1) * 128, oc * 512:(oc + 1) * 512],
                    in_=ost)

            def g_oproj(c, st, oc, pool, tag):
                pout = pool.tile([128, 512], F32, tag=tag, name=f"po{st}_{oc}")
                for ci in range(2):
                    nc.tensor.matmul(pout, lhsT=yt[ci][:, st * 128:(st + 1) * 128],
                                     rhs=wo_sb[ci][:, oc * 512:(oc + 1) * 512],
                                     start=(ci == 0), stop=(ci == 1))
                    yield
                ost = pOst.tile([128, 512], F32, tag="ost", name=f"ost{st}_{oc}")
                nc.vector.tensor_copy(out=ost, in_=pout)
                dma(out=y_d[st * 128:(st + 1) * 128, oc * 512:(oc + 1) * 512],
                    in_=ost)

            # two-priority task queues: prim (py/o_proj, pulled first),
            # sec (K/Q proj with ordering markers)
            prim = deque()
            sec = deque()

            def drain_until(key):
                # run sec tasks until marker `key` has been passed
                while key not in reached:
                    if sec and isinstance(sec[0], str):
                        reached.add(sec.popleft())
                        continue
                    if not sec:
                        return
                    try:
                        next(sec[0])
                    except StopIteration:
                        sec.popleft()

            def drain_all():
                while prim or sec:
                    q = prim if prim else sec
                    obj = q[0]
                    if isinstance(obj, str):
                        q.popleft()
                        continue
                    try:
                        next(obj)
                    except StopIteration:
                        q.popleft()

            rr = [0]
            reached = set()

            def _skip_markers():
                while sec and isinstance(sec[0], str):
                    reached.add(sec.popleft())

            def pull2(n=1, use_prim=True):
                emitted = 0
                while emitted < n:
                    rr[0] ^= 1
                    _skip_markers()
                    qs = [prim, sec] if (rr[0] and use_prim) else [sec, prim]
                    if not use_prim:
                        qs = [sec]
                    q = None
                    for cand in qs:
                        if cand:
                            q = cand
                            break
                    if q is None:
                        return
                    try:
                        next(q[0])
                        emitted += 1
                    except StopIteration:
                        q.popleft()

            # ---- prelude: V proj + K(tt0/tt2) + Q(c0, tt0/tt2), interleaved
            # at group granularity so rope DVE reads overlap the next group ----
            with tc.tile_pool(name="ps_pre", bufs=4, space="PSUM") as pPre:
                def merge(pre, kq_pre):
                    # merge V and K/Q streams ~3 PE instructions at a time so
                    # rope DVE latency hides behind V-proj matmuls
                    while pre or kq_pre:
                        for q in (pre, kq_pre):
                            if not q:
                                continue
                            g = q[0]
                            done = False
                            for _ in range(3):
                                try:
                                    next(g)
                                except StopIteration:
                                    done = True
                                    break
                            if done:
                                q.popleft()

                # phase A: everything that only needs x columns 0..1023
                preA = deque(g_vproj(st) for st in range(8))
                kqA = deque()
                for c in (0, 1):
                    kqA.append(g_proj(wk_sb, kt, 0, c, f"k0{c}", pPre, "pp", pPre, "pp"))
                for c in (0, 1):
                    kqA.append(g_proj(wk_sb, kt, 2, c, f"k2{c}", pPre, "pp", pPre, "pp"))
                kqA.append(g_proj(wq_sb, qt, 0, 0, "q00", pPre, "pp", pPre, "pp"))
                kqA.append(g_proj(wq_sb, qt, 2, 0, "q20", pPre, "pp", pPre, "pp"))
                merge(preA, kqA)
                # phase B: second x half
                preB = deque(g_vproj(st) for st in range(8, NT))
                kqB = deque()
                for c in (2, 3):
                    kqB.append(g_proj(wk_sb, kt, 0, c, f"k0{c}", pPre, "pp", pPre, "pp"))
                for c in (2, 3):
                    kqB.append(g_proj(wk_sb, kt, 2, c, f"k2{c}", pPre, "pp", pPre, "pp"))
                merge(preB, kqB)

            # ---- attention + group norm (prelude PSUM pool closed) ----
            with tc.tile_pool(name="ps_pv", bufs=2, space="PSUM") as pPv, \
                 tc.tile_pool(name="ps_q", bufs=1, space="PSUM") as pPq, \
                 tc.tile_pool(name="ps_perm", bufs=1, space="PSUM") as pPerm, \
                 tc.tile_pool(name="ps_s", bufs=3, space="PSUM") as pPs:
                for c in range(NCH):
                    sec.append(g_proj(wk_sb, kt, 1, c, f"k1{c}", pPq, "psq", pPerm, "perm"))
                for c in range(NCH):
                    sec.append(g_proj(wk_sb, kt, 3, c, f"k3{c}", pPq, "psq", pPerm, "perm"))
                sec.append(g_proj(wq_sb, qt, 1, 0, "q10", pPq, "psq", pPerm, "perm"))
                sec.append(g_proj(wq_sb, qt, 3, 0, "q30", pPq, "psq", pPerm, "perm"))
                sec.append("c0h2")
                for c in range(1, NCH):
                    sec.append(g_proj(wq_sb, qt, 0, c, f"q0{c}", pPq, "psq", pPerm, "perm"))
                    sec.append(g_proj(wq_sb, qt, 2, c, f"q2{c}", pPq, "psq", pPerm, "perm"))
                    sec.append(f"c{c}h0")
                    sec.append(g_proj(wq_sb, qt, 1, c, f"q1{c}", pPq, "psq", pPerm, "perm"))
                    sec.append(g_proj(wq_sb, qt, 3, c, f"q3{c}", pPq, "psq", pPerm, "perm"))
                    sec.append(f"c{c}h2")
                for c in range(NCH):
                    cs = slice(c * 512, (c + 1) * 512)
                    for h in range(HPC):
                        if h == 0 and c > 0:
                            drain_until(f"c{c}h0")
                        if h == 2:
                            drain_until(f"c{c}h2")
                        ro = (h % 2) * 64
                        ows = []
                        for br, tt in ((0, h // 2), (1, 2 + h // 2)):
                            pv = pPv.tile([128, 4, 65], F32, tag="pv",
                                          name=f"pv{c}{h}{br}")
                            prev_e = None
                            for sk in range(NT):
                                pss = pPs.tile([128, 512], F32, tag="pss",
                                               name=f"pss{c}{h}{br}{sk}")
                                nc.tensor.matmul(
                                    pss,
                                    lhsT=kt[tt][ro:ro + 64, sk * 128:(sk + 1) * 128],
                                    rhs=qt[tt][ro:ro + 64, cs],
                                    start=True, stop=True)
                                e = pE.tile([128, 512], BF16, tag="e",
                                            name=f"e{c}{h}{br}{sk}")
                                nc.scalar.activation(out=e, in_=pss, func=AF.Exp,
                                                     scale=QK_SCALE)
                                if prev_e is not None:
                                    va_h = vaug[sk - 1][:, h * 65:(h + 1) * 65]
                                    for qs in range(4):
                                        # start=True marks the whole PSUM bank
                                        # pending-zero, so only the first
                                        # matmul of the bank may set it
                                        nc.tensor.matmul(
                                            pv[:, qs, :],
                                            lhsT=prev_e[:, qs * 128:(qs + 1) * 128],
                                            rhs=va_h,
                                            start=(sk - 1 == 0 and qs == 0),
                                            stop=False)
                                prev_e = e
                                pull2(2, use_prim=(br == 1))
                            va_h = vaug[NT - 1][:, h * 65:(h + 1) * 65]
                            for qs in range(4):
                                nc.tensor.matmul(
                                    pv[:, qs, :],
                                    lhsT=prev_e[:, qs * 128:(qs + 1) * 128],
                                    rhs=va_h, start=False, stop=True)
                            ow = pOw.tile([128, 4, 65], F32, tag="ow",
                                          name=f"ow{c}{h}{br}")
                            nc.vector.tensor_copy(out=ow, in_=pv)
                            ows.append(ow)

                        # group norm for this head (4 s-tiles = the 4 "groups")
                        ow1, ow2 = ows
                        r1 = ow1[:, :, Hd]
                        r2 = ow2[:, :, Hd]
                        rec = pG.tile([128, 4], F32, tag="rec", name=f"rec{c}{h}")
                        nc.vector.reciprocal(rec, r2)
                        rho = pG.tile([128, 4], F32, tag="rho", name=f"rho{c}{h}")
                        nc.vector.tensor_tensor(out=rho, in0=r1, in1=rec, op=MULT)
                        nc.vector.tensor_tensor(out=rho, in0=rho, in1=lam_bh[h], op=MULT)
                        dt_ = pG.tile([128, THDC], F32, tag="dt", name=f"dt{c}{h}")
                        dtv = dt_[:, :].rearrange("p (g d) -> p g d", g=4)
                        nc.vector.tensor_tensor(out=dtv, in0=ow2[:, :, 0:Hd],
                                                in1=_b64(rho), op=MULT)
                        nc.vector.tensor_tensor(out=dtv, in0=ow1[:, :, 0:Hd],
                                                in1=dtv, op=SUB)
                        s1 = pG.tile([128, 4], F32, tag="s1", name=f"s1{c}{h}")
                        nc.vector.reduce_sum(out=s1, in_=dtv, axis=AX)
                        nc.vector.tensor_scalar_mul(s1, s1, -1.0 / Hd)
                        nc.vector.tensor_tensor(out=dtv, in0=dtv, in1=_b64(s1), op=ADD)
                        d2 = pG.tile([128, THDC], F32, tag="d2", name=f"d2{c}{h}")
                        nc.vector.tensor_tensor(out=d2, in0=dt_, in1=dt_, op=MULT)
                        s2 = pG.tile([128, 4], F32, tag="s2", name=f"s2{c}{h}")
                        nc.vector.reduce_sum(
                            out=s2, in_=d2[:, :].rearrange("p (g d) -> p g d", g=4),
                            axis=AX)
                        nc.vector.tensor_tensor(out=rec, in0=r1, in1=r1, op=MULT)
                        nc.vector.tensor_scalar_mul(rec, rec, GN_EPS)
                        nc.vector.tensor_scalar_mul(s2, s2, 1.0 / Hd)
                        nc.vector.tensor_tensor(out=s2, in0=s2, in1=rec, op=ADD)
                        nc.scalar.activation(out=s2, in_=s2, func=AF.Sqrt)
                        nc.vector.reciprocal(s2, s2)
                        nc.vector.tensor_tensor(out=dtv, in0=dtv, in1=_b64(s2), op=MULT)
                        gw = gnw_eff[:, h * Hd:(h + 1) * Hd].unsqueeze(1) \
                            .broadcast_to((128, 4, Hd))
                        gb = gnb_eff[:, h * Hd:(h + 1) * Hd].unsqueeze(1) \
                            .broadcast_to((128, 4, Hd))
                        nc.vector.tensor_tensor(out=dtv, in0=dtv, in1=gw, op=MULT)
                        nc.vector.tensor_tensor(out=dtv, in0=dtv, in1=gb, op=ADD)
                        prim.append(g_py(c, h, dt_, pPerm))
                        if h == HPC - 1 and c < NCH - 1:
                            for stl in range(4):
                                st = c * 4 + stl
                                for oc in range(2):
                                    prim.append(g_oproj(c, st, oc, pVout, "vout"))
                        if h == 1 and c == NCH - 1:
                            for stl in range(4):
                                st = c * 4 + stl
                                for oc in range(2):
                                    prim.append(g_oproj_ci0(st, oc))
                drain_all()
            # pPs closed: 2 banks free for double-buffered tail o_proj
            with tc.tile_pool(name="ps_tail", bufs=2, space="PSUM") as pTail:
                tail = []
                for stl in range(4):
                    st = (NCH - 1) * 4 + stl
                    for oc in range(2):
                        tail.append(g_oproj_ci1(st, oc, pTail, "tail"))
                # interleave pairs of groups so the two PSUM slots overlap
                for i in range(0, len(tail), 2):
                    alive = list(tail[i:i + 2])
                    while alive:
                        for g in list(alive):
                            try:
                                next(g)
                            except StopIteration:
                                alive.remove(g)

    _split_multi_waits(nc)
    return nc


_CACHE = {}


def _get_module():
    if "nc" not in _CACHE:
        _CACHE["nc"] = build_module()
        _CACHE["tables"] = _rope_tables()
    return _CACHE["nc"], _CACHE["tables"]


def kernel(x, Wq, Wk, Wv, Wo, lambda_q1, lambda_k1, lambda_q2, lambda_k2,
           lambda_init, gn_weight, gn_bias):
    from concourse.bass_utils import run_bass_kernel_spmd

    x = np.asarray(x, dtype=np.float32)
    Wq = np.asarray(Wq, dtype=np.float32)
    Wk = np.asarray(Wk, dtype=np.float32)
    Wv = np.asarray(Wv, dtype=np.float32)
    Wo = np.asarray(Wo, dtype=np.float32)
    lq1 = np.asarray(lambda_q1, dtype=np.float32)
    lk1 = np.asarray(lambda_k1, dtype=np.float32)
    lq2 = np.asarray(lambda_q2, dtype=np.float32)
    lk2 = np.asarray(lambda_k2, dtype=np.float32)
    lam_init = np.float32(np.asarray(lambda_init).reshape(()))
    gnw = np.asarray(gn_weight, dtype=np.float32)
    gnb = np.asarray(gn_bias, dtype=np.float32)

    nc, (cosT, ssinT) = _get_module()

    xb = [np.ascontiguousarray(x[b].astype(NPBF16)) for b in range(B)]
    in_maps = []
    for core in range(8):
        b = core // 4
        hb = (core % 4) * HPC
        c1 = slice(hb * Hd, (hb + HPC) * Hd)
        c2 = slice(H * Hd + hb * Hd, H * Hd + (hb + HPC) * Hd)
        lam = np.stack([lq1[hb:hb + HPC], lk1[hb:hb + HPC],
                        lq2[hb:hb + HPC], lk2[hb:hb + HPC],
                        np.full(HPC, lam_init, np.float32)]).astype(np.float32)
        in_maps.append({
            "x": xb[b],
            "wq": np.ascontiguousarray(
                np.concatenate([Wq[:, c1], Wq[:, c2]], axis=1).astype(NPBF16)),
            "wk": np.ascontiguousarray(
                np.concatenate([Wk[:, c1], Wk[:, c2]], axis=1).astype(NPBF16)),
            "wv": np.ascontiguousarray(Wv[:, c1].astype(NPBF16)),
            "wo": np.ascontiguousarray(Wo[c1, :].astype(NPBF16)),
            "lam": lam,
            "gnw": np.ascontiguousarray(gnw[c1]),
            "gnb": np.ascontiguousarray(gnb[c1]),
            "cosT": cosT,
            "ssinT": ssinT,
        })

    last_err = None
    for attempt in range(3):
        try:
            res = run_bass_kernel_spmd(nc, in_maps, core_ids=list(range(8)))
            break
        except Exception as e:  # transient axon/device hiccups
            last_err = e
            time.sleep(10 * (attempt + 1))
    else:
        raise last_err

    out = np.zeros((B, S, D), dtype=np.float32)
    for core in range(8):
        out[core // 4] += res.results[core]["y"]
    return out
